# revision 15
# baseline (speedup 1.0000x reference)
"""TRN2 Bass kernel for nn_Attention_79628693668242 (sparse attention).

Head-parallel (tensor parallel) across 8 NeuronCores, 2 heads per core:
  cores 0-3: heads (2c, 2c+1)            -- both causal self-attention
  cores 4-7: heads (c+4, c+8)            -- one self head + one cross head

Uniform SPMD program; per-core behavior differs only through data:
  - mask tensors (causal staircase vs all-ones)
  - a flag that predicate-selects slot-B attention k/v between the current
    projection (self) and the prev-layer cache (cross)

All matmul operands are kept in layouts that need no transposes:
  - x is host-transposed to xT [B, C, T] (fp32r)
  - q,k projected feature-major [d, t]; v projected token-major [t, d]
  - PV computes yT = v.T @ pT feature-major, which directly feeds the
    output projection as lhsT; proj emits token-major y tiles.
Softmax skips the max-subtraction (scores are bounded ~|6| here, exp is
safe in fp32) which makes the whole softmax free-dim-local except for the
denominator: S = sum_chunks exp-chunks (DVE), column sums via a ones
matmul, reciprocal, gpsimd partition-broadcast, folded into the PV-psum
eviction multiply.
"""

import numpy as np
import ml_dtypes

import concourse.bass as bass
import concourse.tile as tile
from concourse import bacc, mybir
from concourse.bass_utils import run_bass_kernel_spmd

F32 = mybir.dt.float32
F32R = mybir.dt.float32r
BF16 = mybir.dt.bfloat16
AF = mybir.ActivationFunctionType

B, T, C, D = 4, 2048, 2048, 128
H, SA = 16, 12
CK = C // 128          # 16 contraction chunks
TK = T // 128          # 16 token chunks per batch
W = 512                # q / t window
NW = T // W            # 4 windows per batch
MCOLS = 1152           # mask columns: slice offsets 0..640

N_CORES = 8


def _core_heads(c):
    if c < 4:
        return 2 * c, 2 * c + 1
    return c + 4, c + 8


def _emit(nc):
    xT = nc.declare_dram_parameter("xT", [B, C, T], F32R, isOutput=False)
    wqkv = nc.declare_dram_parameter("wqkv", [C, 6, 128], F32R, isOutput=False)
    wproj = nc.declare_dram_parameter("wproj", [2, 128, C], BF16, isOutput=False)
    prevKT = nc.declare_dram_parameter("prevKT", [B, 128, T], BF16, isOutput=False)
    prevV = nc.declare_dram_parameter("prevV", [B, T, 128], BF16, isOutput=False)
    maskA = nc.declare_dram_parameter("maskA", [128, MCOLS], BF16, isOutput=False)
    maskB = nc.declare_dram_parameter("maskB", [128, MCOLS], BF16, isOutput=False)
    flagB = nc.declare_dram_parameter("flagB", [128, 1], mybir.dt.uint8, isOutput=False)
    flagF = nc.declare_dram_parameter("flagF", [128, 1], F32, isOutput=False)
    onesP = nc.declare_dram_parameter("onesP", [128, 1], F32R, isOutput=False)
    flagI = nc.declare_dram_parameter("flagI", [128, 1], F32, isOutput=False)

    y_out = nc.declare_dram_parameter("y_out", [B, C, T], F32, isOutput=True)
    curk = nc.declare_dram_parameter("curk", [B, 2, 128, T], F32R, isOutput=True)
    curv = nc.declare_dram_parameter("curv", [B, 2, T, 128], F32, isOutput=True)

    with tile.TileContext(nc) as tc:
        with (
            tc.tile_pool(name="const", bufs=1) as const,
            tc.tile_pool(name="xwp", bufs=2) as xwp,
            tc.tile_pool(name="qkvp", bufs=1) as qkvp,
            tc.tile_pool(name="attn", bufs=1) as attnp,
            tc.tile_pool(name="yw", bufs=2) as ywp,
            tc.tile_pool(name="prevp", bufs=1) as prevp,
            tc.tile_pool(name="ps_a", bufs=3, space="PSUM") as ps_a,
            tc.tile_pool(name="ps_y", bufs=2, space="PSUM") as ps_y,
            tc.tile_pool(name="ps_v", bufs=2, space="PSUM") as ps_v,
            tc.tile_pool(name="ps_d", bufs=1, space="PSUM") as ps_d,
        ):
            # ---- persistent loads ----
            wq_sb = const.tile([128, CK, 6, 128], F32R)
            nc.sync.dma_start(
                wq_sb[:], wqkv.rearrange("(ck p) s d -> p ck s d", p=128)
            )
            wp_sb = const.tile([128, 2, C], BF16)
            nc.sync.dma_start(wp_sb[:], wproj.rearrange("h p o -> p h o"))
            mA_sb = const.tile([128, MCOLS], BF16)
            nc.sync.dma_start(mA_sb[:], maskA[:, :])
            mB_sb = const.tile([128, MCOLS], BF16)
            nc.sync.dma_start(mB_sb[:], maskB[:, :])
            flag_sb = const.tile([128, 1], mybir.dt.uint8)
            nc.sync.dma_start(flag_sb[:], flagB[:, :])
            flagf_sb = const.tile([128, 1], F32)
            nc.sync.dma_start(flagf_sb[:], flagF[:, :])
            flagi_sb = const.tile([128, 1], F32)
            nc.sync.dma_start(flagi_sb[:], flagI[:, :])
            ones_sb = const.tile([128, 1], F32)
            nc.vector.memset(ones_sb[:], 1.0)

            masks = (mA_sb, mB_sb)

            for b in range(B):
                # ---- QKV projection for batch b ----
                qT = qkvp.tile([128, 2, T], F32R, tag="qT")
                kT = qkvp.tile([128, 2, T], F32R, tag="kT")
                vat = [
                    qkvp.tile([128, TK, 128], BF16, tag=f"v{s}", name=f"vat{s}")
                    for s in (0, 1)
                ]
                for tw in range(NW):
                    xw = xwp.tile([128, CK, W], F32R, tag="xw")
                    nc.sync.dma_start(
                        xw[:],
                        xT[b].rearrange("(ck p) t -> p ck t", p=128)[
                            :, :, tw * W : (tw + 1) * W
                        ],
                    )
                    # q/k feature-major: out [f 128, t W]
                    for s in range(4):  # qA qB kA kB
                        acc = ps_a.tile([128, W], F32, tag="mm512")
                        for ck in range(CK):
                            nc.tensor.matmul(
                                acc[:],
                                wq_sb[:, ck, s, :],
                                xw[:, ck, :],
                                start=(ck == 0),
                                stop=(ck == CK - 1),
                            )
                        dst = qT if s < 2 else kT
                        nc.vector.tensor_copy(
                            dst[:, s % 2, tw * W : (tw + 1) * W], acc[:]
                        )
                    # v token-major: out [t 128, (slot, d) 256]
                    for sub in range(W // 128):
                        ckg = tw * (W // 128) + sub
                        accv = ps_v.tile([128, 2, 128], F32, tag="vmm")
                        for ck in range(CK):
                            nc.tensor.matmul(
                                accv[:],
                                xw[:, ck, sub * 128 : (sub + 1) * 128],
                                wq_sb[:, ck, 4:6, :],
                                start=(ck == 0),
                                stop=(ck == CK - 1),
                            )
                        vstage = ywp.tile([128, 2, 128], F32, tag="ystage")
                        for s in (0, 1):
                            nc.vector.tensor_copy(
                                vat[s][:, ckg, :], accv[:, s, :]
                            )
                            nc.vector.tensor_copy(vstage[:, s, :], accv[:, s, :])
                            nc.sync.dma_start(
                                curv[b, s, ckg * 128 : (ckg + 1) * 128, :],
                                vstage[:, s, :],
                            )

                # cur_k out (feature-major; host transposes)
                for s in (0, 1):
                    nc.sync.dma_start(curk[b, s], kT[:, s, :])

                # ---- slot-B attention k/v select (self proj vs prev cache) ----
                pk_sb = prevp.tile([128, T], BF16, tag="pk")
                nc.sync.dma_start(pk_sb[:], prevKT[b])
                # kT_B = kT_B * (1-flag) + prev * flag   (fp32r-safe blend)
                nc.vector.tensor_scalar_mul(kT[:, 1, :], kT[:, 1, :], flagi_sb[:])
                nc.vector.tensor_scalar_mul(pk_sb[:], pk_sb[:], flagf_sb[:])
                nc.vector.tensor_tensor(
                    kT[:, 1, :], kT[:, 1, :], pk_sb[:], mybir.AluOpType.add
                )
                pv_sb = prevp.tile([128, TK, 128], BF16, tag="pv")
                nc.sync.dma_start(
                    pv_sb[:], prevV[b].rearrange("(ck p) d -> p ck d", p=128)
                )
                nc.vector.copy_predicated(
                    vat[1][:], flag_sb[:].to_broadcast((128, TK, 128)), pv_sb[:]
                )

                # ---- attention + projection, per window ----
                yT = qkvp.tile([128, 2, T], BF16, tag="yT")
                for wp2 in range(NW // 2):
                  for w in (2 * wp2, 2 * wp2 + 1):
                    yTw = yT[:, :, w * W : (w + 1) * W]
                    for slot in (0, 1):
                        nch = (4 * w + 4) if slot == 0 else TK
                        pT = attnp.tile([128, TK, W], BF16, tag="pT")
                        S = attnp.tile([128, W], F32, tag="S")
                        for j in range(nch):
                            accs = ps_a.tile([128, W], F32, tag="mm512")
                            nc.tensor.matmul(
                                accs[:],
                                kT[:, slot, j * 128 : (j + 1) * 128],
                                qT[:, slot, w * W : (w + 1) * W],
                                start=True,
                                stop=True,
                            )
                            nc.scalar.activation(pT[:, j, :], accs[:], AF.Exp)
                            r = min(max(j - 4 * w, -1), 4)
                            if r >= 0:
                                off = 512 - 128 * r
                                nc.gpsimd.tensor_tensor(
                                    pT[:, j, :],
                                    pT[:, j, :],
                                    masks[slot][:, off : off + W],
                                    mybir.AluOpType.mult,
                                )
                            if j == 0:
                                nc.any.tensor_copy(S[:], pT[:, 0, :])
                            else:
                                nc.any.tensor_tensor(
                                    S[:], S[:], pT[:, j, :], mybir.AluOpType.add
                                )
                        # PV: yT [d 128, q W] accumulated over k chunks
                        accy = ps_y.tile([128, W], F32, tag="pv512")
                        for j in range(nch):
                            nc.tensor.matmul(
                                accy[:],
                                vat[slot][:, j, :],
                                pT[:, j, :],
                                start=(j == 0),
                                stop=(j == nch - 1),
                            )
                        # denominator: column sums of S via ones matmul
                        dps = ps_d.tile([1, W], F32, tag="den")
                        nc.tensor.matmul(
                            dps[:], ones_sb[:], S[:], start=True, stop=True
                        )
                        recip = attnp.tile([1, W], F32, tag="recip")
                        nc.vector.reciprocal(recip[:], dps[:])
                        recipb = attnp.tile([128, W], F32, tag="recipb")
                        nc.gpsimd.partition_broadcast(recipb[:], recip[:])
                        nc.vector.tensor_tensor(
                            yTw[:, slot, :], accy[:], recipb[:], mybir.AluOpType.mult
                        )

                  # output projection for this window pair, feature-major out
                  for oc in range(C // 128):
                        accps = [
                            ps_a.tile([128, W], F32, tag="mm512", name=f"accp{i}")
                            for i in range(2)
                        ]
                        for slot in (0, 1):
                            for i in range(2):
                                w = wp2 * 2 + i
                                nc.tensor.matmul(
                                    accps[i][:],
                                    wp_sb[:, slot, oc * 128 : (oc + 1) * 128],
                                    yT[:, slot, w * W : (w + 1) * W],
                                    start=(slot == 0),
                                    stop=(slot == 1),
                                )
                        for i in range(2):
                            w = wp2 * 2 + i
                            ystage = ywp.tile([128, W], F32, tag="ystage")
                            nc.vector.tensor_copy(ystage[:], accps[i][:])
                            nc.sync.dma_start(
                                y_out[b, oc * 128 : (oc + 1) * 128,
                                      w * W : (w + 1) * W],
                                ystage[:],
                            )
    nc.compile()
    return nc


_CACHE = {}


def _get_nc():
    if "nc" not in _CACHE:
        nc = bacc.Bacc(None, target_bir_lowering=False, debug=False)
        _CACHE["nc"] = _emit(nc)
    return _CACHE["nc"]


def _host_inputs(x, w_attn, w_proj, prevs):
    """Build the 8 per-core input maps."""
    xTh = np.ascontiguousarray(x.transpose(0, 2, 1), dtype=np.float32)  # [B,C,T]
    scale = np.float32(1.0 / np.sqrt(D))

    kk = np.arange(128)[:, None]
    cc = np.arange(MCOLS)[None, :]
    stair = (cc >= kk + 512).astype(ml_dtypes.bfloat16)
    ones_m = np.ones((128, MCOLS), dtype=ml_dtypes.bfloat16)
    zkt = np.zeros((B, 128, T), dtype=ml_dtypes.bfloat16)
    zv = np.zeros((B, T, 128), dtype=ml_dtypes.bfloat16)

    in_maps = []
    for c in range(N_CORES):
        hA, hB = _core_heads(c)
        cols = []
        for h in (hA, hB):
            cols.append(w_attn[h * D : (h + 1) * D, :].T * scale)  # q (scaled)
        for h in (hA, hB):
            cols.append(w_attn[C + h * D : C + (h + 1) * D, :].T)  # k
        for h in (hA, hB):
            cols.append(w_attn[2 * C + h * D : 2 * C + (h + 1) * D, :].T)  # v
        # order: qA qB kA kB vA vB -> [C, 6, 128]
        wq = np.stack(cols, axis=1).astype(np.float32)
        wp = np.stack(
            [w_proj[:, hA * D : (hA + 1) * D].T, w_proj[:, hB * D : (hB + 1) * D].T],
            axis=0,
        ).astype(ml_dtypes.bfloat16)  # [2, 128, C]

        cross = c >= 4
        if cross:
            i = hB - SA
            pk, pv = prevs[i]
            pkt = np.ascontiguousarray(pk[:, hB].transpose(0, 2, 1)).astype(
                ml_dtypes.bfloat16
            )  # [B,128,T]
            pvv = np.ascontiguousarray(pv[:, hB]).astype(ml_dtypes.bfloat16)
        else:
            pkt, pvv = zkt, zv

        in_maps.append(
            {
                "xT": xTh,
                "wqkv": np.ascontiguousarray(wq),
                "wproj": np.ascontiguousarray(wp),
                "prevKT": pkt,
                "prevV": pvv,
                "maskA": stair,
                "maskB": ones_m if cross else stair,
                "flagB": np.full((128, 1), 1 if cross else 0, dtype=np.uint8),
                "flagF": np.full((128, 1), 1.0 if cross else 0.0, dtype=np.float32),
                "onesP": np.ones((128, 1), dtype=np.float32),
                "flagI": np.full((128, 1), 0.0 if cross else 1.0, dtype=np.float32),
            }
        )
    return in_maps


def kernel(x, w_attn, w_proj,
           prev_k0, prev_v0, prev_k1, prev_v1,
           prev_k2, prev_v2, prev_k3, prev_v3,
           _trace=False):
    x = np.asarray(x, dtype=np.float32)
    w_attn = np.asarray(w_attn, dtype=np.float32)
    w_proj = np.asarray(w_proj, dtype=np.float32)
    prevs = [
        (np.asarray(prev_k0), np.asarray(prev_v0)),
        (np.asarray(prev_k1), np.asarray(prev_v1)),
        (np.asarray(prev_k2), np.asarray(prev_v2)),
        (np.asarray(prev_k3), np.asarray(prev_v3)),
    ]
    nc = _get_nc()
    in_maps = _host_inputs(x, w_attn, w_proj, prevs)
    res = run_bass_kernel_spmd(
        nc, in_maps, core_ids=list(range(N_CORES)), trace=_trace
    )
    kernel.last_exec_time_ns = res.exec_time_ns

    y = np.zeros((B, C, T), dtype=np.float64)
    cur_k = np.zeros((B, H, T, D), dtype=np.float32)
    cur_v = np.zeros((B, H, T, D), dtype=np.float32)
    for c in range(N_CORES):
        out = res.results[c]
        y += out["y_out"].astype(np.float64)
        hA, hB = _core_heads(c)
        for s, h in ((0, hA), (1, hB)):
            cur_k[:, h] = out["curk"][:, s].transpose(0, 2, 1)
            cur_v[:, h] = out["curv"][:, s]
    return np.ascontiguousarray(y.transpose(0, 2, 1)).astype(np.float32), cur_k, cur_v


kernel.last_exec_time_ns = None


# revision 16
# speedup vs baseline: 1.3373x; 1.3373x over previous
"""TRN2 Bass kernel for nn_Attention_79628693668242 (sparse attention).

Head-parallel (tensor parallel) across 8 NeuronCores, 2 heads per core:
  cores 0-3: heads (2c, 2c+1)            -- both causal self-attention
  cores 4-7: heads (c+4, c+8)            -- one self head + one cross head

Uniform SPMD program; per-core behavior differs only through data:
  - mask tensors (causal staircase vs all-ones)
  - a flag that predicate-selects slot-B attention k/v between the current
    projection (self) and the prev-layer cache (cross)

All matmul operands are kept in layouts that need no transposes:
  - x is host-transposed to xT [B, C, T] (fp32r)
  - q,k projected feature-major [d, t]; v projected token-major [t, d]
  - PV computes yT = v.T @ pT feature-major, which directly feeds the
    output projection as lhsT; proj emits token-major y tiles.
Softmax skips the max-subtraction (scores are bounded ~|6| here, exp is
safe in fp32) which makes the whole softmax free-dim-local except for the
denominator: S = sum_chunks exp-chunks (DVE), column sums via a ones
matmul, reciprocal, gpsimd partition-broadcast, folded into the PV-psum
eviction multiply.
"""

import numpy as np
import ml_dtypes

import concourse.bass as bass
import concourse.tile as tile
from concourse import bacc, mybir
from concourse.bass_utils import run_bass_kernel_spmd

F32 = mybir.dt.float32
F32R = mybir.dt.float32r
BF16 = mybir.dt.bfloat16
AF = mybir.ActivationFunctionType

B, T, C, D = 4, 2048, 2048, 128
H, SA = 16, 12
CK = C // 128          # 16 contraction chunks
TK = T // 128          # 16 token chunks per batch
W = 512                # q / t window
NW = T // W            # 4 windows per batch
MCOLS = 1152           # mask columns: slice offsets 0..640

N_CORES = 8


def _core_heads(c):
    if c < 4:
        return 2 * c, 2 * c + 1
    return c + 4, c + 8


def _emit(nc):
    xT = nc.declare_dram_parameter("xT", [B, C, T], F32R, isOutput=False)
    wqkv = nc.declare_dram_parameter("wqkv", [C, 6, 128], F32R, isOutput=False)
    wproj = nc.declare_dram_parameter("wproj", [2, 128, C], BF16, isOutput=False)
    prevKT = nc.declare_dram_parameter("prevKT", [B, 128, T], BF16, isOutput=False)
    prevV = nc.declare_dram_parameter("prevV", [B, T, 128], BF16, isOutput=False)
    maskA = nc.declare_dram_parameter("maskA", [128, MCOLS], BF16, isOutput=False)
    maskB = nc.declare_dram_parameter("maskB", [128, MCOLS], BF16, isOutput=False)
    flagB = nc.declare_dram_parameter("flagB", [128, 1], mybir.dt.uint8, isOutput=False)
    flagF = nc.declare_dram_parameter("flagF", [128, 1], F32, isOutput=False)
    onesP = nc.declare_dram_parameter("onesP", [128, 1], F32R, isOutput=False)
    flagI = nc.declare_dram_parameter("flagI", [128, 1], F32, isOutput=False)

    y_out = nc.declare_dram_parameter("y_out", [B, C, T], F32, isOutput=True)
    curk = nc.declare_dram_parameter("curk", [B, 2, 128, T], F32R, isOutput=True)
    curv = nc.declare_dram_parameter("curv", [B, 2, T, 128], F32, isOutput=True)

    with tile.TileContext(nc) as tc:
        with (
            tc.tile_pool(name="const", bufs=1) as const,
            tc.tile_pool(name="xwp", bufs=2) as xwp,
            tc.tile_pool(name="qkvp", bufs=1) as qkvp,
            tc.tile_pool(name="attn", bufs=1) as attnp,
            tc.tile_pool(name="yw", bufs=2) as ywp,
            tc.tile_pool(name="prevp", bufs=1) as prevp,
            tc.tile_pool(name="ps_a", bufs=3, space="PSUM") as ps_a,
            tc.tile_pool(name="ps_y", bufs=2, space="PSUM") as ps_y,
            tc.tile_pool(name="ps_v", bufs=2, space="PSUM") as ps_v,
            tc.tile_pool(name="ps_d", bufs=1, space="PSUM") as ps_d,
        ):
            # ---- persistent loads ----
            wq_sb = const.tile([128, CK, 6, 128], F32R)
            nc.sync.dma_start(
                wq_sb[:], wqkv.rearrange("(ck p) s d -> p ck s d", p=128)
            )
            wp_sb = const.tile([128, 2, C], BF16)
            nc.sync.dma_start(wp_sb[:], wproj.rearrange("h p o -> p h o"))
            mA_sb = const.tile([128, MCOLS], BF16)
            nc.sync.dma_start(mA_sb[:], maskA[:, :])
            mB_sb = const.tile([128, MCOLS], BF16)
            nc.sync.dma_start(mB_sb[:], maskB[:, :])
            flag_sb = const.tile([128, 1], mybir.dt.uint8)
            nc.sync.dma_start(flag_sb[:], flagB[:, :])
            flagf_sb = const.tile([128, 1], F32)
            nc.sync.dma_start(flagf_sb[:], flagF[:, :])
            flagi_sb = const.tile([128, 1], F32)
            nc.sync.dma_start(flagi_sb[:], flagI[:, :])
            ones_sb = const.tile([128, 1], BF16)
            nc.vector.memset(ones_sb[:], 1.0)

            masks = (mA_sb, mB_sb)

            for b in range(B):
                # ---- QKV projection for batch b ----
                qT = qkvp.tile([128, 2, T], F32R, tag="qT")
                kT = qkvp.tile([128, 2, T], F32R, tag="kT")
                vat = [
                    qkvp.tile([128, TK, 128], BF16, tag=f"v{s}", name=f"vat{s}")
                    for s in (0, 1)
                ]
                for tw in range(NW):
                    xw = xwp.tile([128, CK, W], F32R, tag="xw")
                    nc.sync.dma_start(
                        xw[:],
                        xT[b].rearrange("(ck p) t -> p ck t", p=128)[
                            :, :, tw * W : (tw + 1) * W
                        ],
                    )
                    # q/k feature-major: out [f 128, t W]
                    for s in range(4):  # qA qB kA kB
                        acc = ps_a.tile([128, W], F32, tag="mm512")
                        for ck in range(CK):
                            nc.tensor.matmul(
                                acc[:],
                                wq_sb[:, ck, s, :],
                                xw[:, ck, :],
                                start=(ck == 0),
                                stop=(ck == CK - 1),
                            )
                        dst = qT if s < 2 else kT
                        nc.vector.tensor_copy(
                            dst[:, s % 2, tw * W : (tw + 1) * W], acc[:]
                        )
                    # v token-major: out [t 128, (slot, d) 256]
                    for sub in range(W // 128):
                        ckg = tw * (W // 128) + sub
                        accv = ps_v.tile([128, 2, 128], F32, tag="vmm")
                        for ck in range(CK):
                            nc.tensor.matmul(
                                accv[:],
                                xw[:, ck, sub * 128 : (sub + 1) * 128],
                                wq_sb[:, ck, 4:6, :],
                                start=(ck == 0),
                                stop=(ck == CK - 1),
                            )
                        vstage = ywp.tile([128, 2, 128], F32, tag="ystage")
                        for s in (0, 1):
                            nc.vector.tensor_copy(
                                vat[s][:, ckg, :], accv[:, s, :]
                            )
                            nc.vector.tensor_copy(vstage[:, s, :], accv[:, s, :])
                            nc.sync.dma_start(
                                curv[b, s, ckg * 128 : (ckg + 1) * 128, :],
                                vstage[:, s, :],
                            )

                # cur_k out (feature-major; host transposes)
                for s in (0, 1):
                    nc.sync.dma_start(curk[b, s], kT[:, s, :])

                # ---- slot-B attention k/v select (self proj vs prev cache) ----
                pk_sb = prevp.tile([128, T], BF16, tag="pk")
                nc.sync.dma_start(pk_sb[:], prevKT[b])
                # kT_B = kT_B * (1-flag) + prev * flag   (fp32r-safe blend)
                nc.vector.tensor_scalar_mul(kT[:, 1, :], kT[:, 1, :], flagi_sb[:])
                nc.vector.tensor_scalar_mul(pk_sb[:], pk_sb[:], flagf_sb[:])
                nc.vector.tensor_tensor(
                    kT[:, 1, :], kT[:, 1, :], pk_sb[:], mybir.AluOpType.add
                )
                pv_sb = prevp.tile([128, TK, 128], BF16, tag="pv")
                nc.sync.dma_start(
                    pv_sb[:], prevV[b].rearrange("(ck p) d -> p ck d", p=128)
                )
                nc.vector.copy_predicated(
                    vat[1][:], flag_sb[:].to_broadcast((128, TK, 128)), pv_sb[:]
                )

                # ---- attention + projection, per window ----
                yT = qkvp.tile([128, 2, T], BF16, tag="yT")
                for wp2 in range(NW // 2):
                  for w in (2 * wp2, 2 * wp2 + 1):
                    yTw = yT[:, :, w * W : (w + 1) * W]
                    for slot in (0, 1):
                        nch = (4 * w + 4) if slot == 0 else TK
                        pT = attnp.tile([128, TK, W], BF16, tag="pT")
                        for j in range(nch):
                            accs = ps_a.tile([128, W], F32, tag="mm512")
                            nc.tensor.matmul(
                                accs[:],
                                kT[:, slot, j * 128 : (j + 1) * 128],
                                qT[:, slot, w * W : (w + 1) * W],
                                start=True,
                                stop=True,
                            )
                            nc.scalar.activation(pT[:, j, :], accs[:], AF.Exp)
                            r = min(max(j - 4 * w, -1), 4)
                            if r >= 0:
                                off = 512 - 128 * r
                                nc.vector.tensor_tensor(
                                    pT[:, j, :],
                                    pT[:, j, :],
                                    masks[slot][:, off : off + W],
                                    mybir.AluOpType.mult,
                                )
                        # denominator: per-chunk ones matmuls into psum
                        dps = ps_d.tile([1, W], F32, tag="den")
                        for j in range(nch):
                            nc.tensor.matmul(
                                dps[:],
                                ones_sb[:],
                                pT[:, j, :],
                                start=(j == 0),
                                stop=(j == nch - 1),
                            )
                        recip = attnp.tile([1, W], F32, tag="recip")
                        nc.vector.reciprocal(recip[:], dps[:])
                        recipb = attnp.tile([128, W], F32, tag="recipb")
                        nc.gpsimd.partition_broadcast(recipb[:], recip[:])
                        # PV: yT [d 128, q W] accumulated over k chunks
                        accy = ps_y.tile([128, W], F32, tag="pv512")
                        for j in range(nch):
                            nc.tensor.matmul(
                                accy[:],
                                vat[slot][:, j, :],
                                pT[:, j, :],
                                start=(j == 0),
                                stop=(j == nch - 1),
                            )
                        nc.vector.tensor_tensor(
                            yTw[:, slot, :], accy[:], recipb[:], mybir.AluOpType.mult
                        )

                  # output projection for this window pair, feature-major out
                  for oc in range(C // 128):
                        accps = [
                            ps_a.tile([128, W], F32, tag="mm512", name=f"accp{i}")
                            for i in range(2)
                        ]
                        for slot in (0, 1):
                            for i in range(2):
                                w = wp2 * 2 + i
                                nc.tensor.matmul(
                                    accps[i][:],
                                    wp_sb[:, slot, oc * 128 : (oc + 1) * 128],
                                    yT[:, slot, w * W : (w + 1) * W],
                                    start=(slot == 0),
                                    stop=(slot == 1),
                                )
                        for i in range(2):
                            w = wp2 * 2 + i
                            ystage = ywp.tile([128, W], F32, tag="ystage")
                            nc.vector.tensor_copy(ystage[:], accps[i][:])
                            nc.sync.dma_start(
                                y_out[b, oc * 128 : (oc + 1) * 128,
                                      w * W : (w + 1) * W],
                                ystage[:],
                            )
    nc.compile()
    return nc


_CACHE = {}


def _get_nc():
    if "nc" not in _CACHE:
        nc = bacc.Bacc(None, target_bir_lowering=False, debug=False)
        _CACHE["nc"] = _emit(nc)
    return _CACHE["nc"]


def _host_inputs(x, w_attn, w_proj, prevs):
    """Build the 8 per-core input maps."""
    xTh = np.ascontiguousarray(x.transpose(0, 2, 1), dtype=np.float32)  # [B,C,T]
    scale = np.float32(1.0 / np.sqrt(D))

    kk = np.arange(128)[:, None]
    cc = np.arange(MCOLS)[None, :]
    stair = (cc >= kk + 512).astype(ml_dtypes.bfloat16)
    ones_m = np.ones((128, MCOLS), dtype=ml_dtypes.bfloat16)
    zkt = np.zeros((B, 128, T), dtype=ml_dtypes.bfloat16)
    zv = np.zeros((B, T, 128), dtype=ml_dtypes.bfloat16)

    in_maps = []
    for c in range(N_CORES):
        hA, hB = _core_heads(c)
        cols = []
        for h in (hA, hB):
            cols.append(w_attn[h * D : (h + 1) * D, :].T * scale)  # q (scaled)
        for h in (hA, hB):
            cols.append(w_attn[C + h * D : C + (h + 1) * D, :].T)  # k
        for h in (hA, hB):
            cols.append(w_attn[2 * C + h * D : 2 * C + (h + 1) * D, :].T)  # v
        # order: qA qB kA kB vA vB -> [C, 6, 128]
        wq = np.stack(cols, axis=1).astype(np.float32)
        wp = np.stack(
            [w_proj[:, hA * D : (hA + 1) * D].T, w_proj[:, hB * D : (hB + 1) * D].T],
            axis=0,
        ).astype(ml_dtypes.bfloat16)  # [2, 128, C]

        cross = c >= 4
        if cross:
            i = hB - SA
            pk, pv = prevs[i]
            pkt = np.ascontiguousarray(pk[:, hB].transpose(0, 2, 1)).astype(
                ml_dtypes.bfloat16
            )  # [B,128,T]
            pvv = np.ascontiguousarray(pv[:, hB]).astype(ml_dtypes.bfloat16)
        else:
            pkt, pvv = zkt, zv

        in_maps.append(
            {
                "xT": xTh,
                "wqkv": np.ascontiguousarray(wq),
                "wproj": np.ascontiguousarray(wp),
                "prevKT": pkt,
                "prevV": pvv,
                "maskA": stair,
                "maskB": ones_m if cross else stair,
                "flagB": np.full((128, 1), 1 if cross else 0, dtype=np.uint8),
                "flagF": np.full((128, 1), 1.0 if cross else 0.0, dtype=np.float32),
                "onesP": np.ones((128, 1), dtype=np.float32),
                "flagI": np.full((128, 1), 0.0 if cross else 1.0, dtype=np.float32),
            }
        )
    return in_maps


def kernel(x, w_attn, w_proj,
           prev_k0, prev_v0, prev_k1, prev_v1,
           prev_k2, prev_v2, prev_k3, prev_v3,
           _trace=False):
    x = np.asarray(x, dtype=np.float32)
    w_attn = np.asarray(w_attn, dtype=np.float32)
    w_proj = np.asarray(w_proj, dtype=np.float32)
    prevs = [
        (np.asarray(prev_k0), np.asarray(prev_v0)),
        (np.asarray(prev_k1), np.asarray(prev_v1)),
        (np.asarray(prev_k2), np.asarray(prev_v2)),
        (np.asarray(prev_k3), np.asarray(prev_v3)),
    ]
    nc = _get_nc()
    in_maps = _host_inputs(x, w_attn, w_proj, prevs)
    res = run_bass_kernel_spmd(
        nc, in_maps, core_ids=list(range(N_CORES)), trace=_trace
    )
    kernel.last_exec_time_ns = res.exec_time_ns

    y = np.zeros((B, C, T), dtype=np.float64)
    cur_k = np.zeros((B, H, T, D), dtype=np.float32)
    cur_v = np.zeros((B, H, T, D), dtype=np.float32)
    for c in range(N_CORES):
        out = res.results[c]
        y += out["y_out"].astype(np.float64)
        hA, hB = _core_heads(c)
        for s, h in ((0, hA), (1, hB)):
            cur_k[:, h] = out["curk"][:, s].transpose(0, 2, 1)
            cur_v[:, h] = out["curv"][:, s]
    return np.ascontiguousarray(y.transpose(0, 2, 1)).astype(np.float32), cur_k, cur_v


kernel.last_exec_time_ns = None


# revision 17
# speedup vs baseline: 1.3457x; 1.0063x over previous
"""TRN2 Bass kernel for nn_Attention_79628693668242 (sparse attention).

Head-parallel (tensor parallel) across 8 NeuronCores, 2 heads per core:
  cores 0-3: heads (2c, 2c+1)            -- both causal self-attention
  cores 4-7: heads (c+4, c+8)            -- one self head + one cross head

Uniform SPMD program; per-core behavior differs only through data:
  - mask tensors (causal staircase vs all-ones)
  - a flag that predicate-selects slot-B attention k/v between the current
    projection (self) and the prev-layer cache (cross)

All matmul operands are kept in layouts that need no transposes:
  - x is host-transposed to xT [B, C, T] (fp32r)
  - q,k projected feature-major [d, t]; v projected token-major [t, d]
  - PV computes yT = v.T @ pT feature-major, which directly feeds the
    output projection as lhsT; proj emits token-major y tiles.
Softmax skips the max-subtraction (scores are bounded ~|6| here, exp is
safe in fp32) which makes the whole softmax free-dim-local except for the
denominator: S = sum_chunks exp-chunks (DVE), column sums via a ones
matmul, reciprocal, gpsimd partition-broadcast, folded into the PV-psum
eviction multiply.
"""

import numpy as np
import ml_dtypes

import concourse.bass as bass
import concourse.tile as tile
from concourse import bacc, mybir
from concourse.bass_utils import run_bass_kernel_spmd

F32 = mybir.dt.float32
F32R = mybir.dt.float32r
BF16 = mybir.dt.bfloat16
AF = mybir.ActivationFunctionType

B, T, C, D = 4, 2048, 2048, 128
H, SA = 16, 12
CK = C // 128          # 16 contraction chunks
TK = T // 128          # 16 token chunks per batch
W = 512                # q / t window
NW = T // W            # 4 windows per batch
MCOLS = 1152           # mask columns: slice offsets 0..640

N_CORES = 8


def _core_heads(c):
    if c < 4:
        return 2 * c, 2 * c + 1
    return c + 4, c + 8


def _emit(nc):
    xT = nc.declare_dram_parameter("xT", [B, C, T], BF16, isOutput=False)
    wqkv = nc.declare_dram_parameter("wqkv", [C, 6, 128], BF16, isOutput=False)
    wproj = nc.declare_dram_parameter("wproj", [2, 128, C], BF16, isOutput=False)
    prevKT = nc.declare_dram_parameter("prevKT", [B, 128, T], BF16, isOutput=False)
    prevV = nc.declare_dram_parameter("prevV", [B, T, 128], BF16, isOutput=False)
    maskA = nc.declare_dram_parameter("maskA", [128, MCOLS], BF16, isOutput=False)
    maskB = nc.declare_dram_parameter("maskB", [128, MCOLS], BF16, isOutput=False)
    flagB = nc.declare_dram_parameter("flagB", [128, 1], mybir.dt.uint8, isOutput=False)
    flagF = nc.declare_dram_parameter("flagF", [128, 1], F32, isOutput=False)
    onesP = nc.declare_dram_parameter("onesP", [128, 1], F32R, isOutput=False)
    flagI = nc.declare_dram_parameter("flagI", [128, 1], F32, isOutput=False)

    y_out = nc.declare_dram_parameter("y_out", [B, C, T], F32, isOutput=True)
    curk = nc.declare_dram_parameter("curk", [B, 2, 128, T], F32R, isOutput=True)
    curv = nc.declare_dram_parameter("curv", [B, 2, T, 128], F32, isOutput=True)

    with tile.TileContext(nc) as tc:
        with (
            tc.tile_pool(name="const", bufs=1) as const,
            tc.tile_pool(name="xwp", bufs=3) as xwp,
            tc.tile_pool(name="qkvp", bufs=1) as qkvp,
            tc.tile_pool(name="attn", bufs=1) as attnp,
            tc.tile_pool(name="ptp", bufs=2) as ptp,
            tc.tile_pool(name="yw", bufs=2) as ywp,
            tc.tile_pool(name="prevp", bufs=1) as prevp,
            tc.tile_pool(name="ps_a", bufs=3, space="PSUM") as ps_a,
            tc.tile_pool(name="ps_y", bufs=2, space="PSUM") as ps_y,
            tc.tile_pool(name="ps_v", bufs=2, space="PSUM") as ps_v,
            tc.tile_pool(name="ps_d", bufs=1, space="PSUM") as ps_d,
        ):
            # ---- persistent loads ----
            wq_sb = const.tile([128, CK, 6, 128], BF16)
            nc.sync.dma_start(
                wq_sb[:], wqkv.rearrange("(ck p) s d -> p ck s d", p=128)
            )
            wp_sb = const.tile([128, 2, C], BF16)
            nc.sync.dma_start(wp_sb[:], wproj.rearrange("h p o -> p h o"))
            mA_sb = const.tile([128, MCOLS], BF16)
            nc.sync.dma_start(mA_sb[:], maskA[:, :])
            mB_sb = const.tile([128, MCOLS], BF16)
            nc.sync.dma_start(mB_sb[:], maskB[:, :])
            flag_sb = const.tile([128, 1], mybir.dt.uint8)
            nc.sync.dma_start(flag_sb[:], flagB[:, :])
            flagf_sb = const.tile([128, 1], F32)
            nc.sync.dma_start(flagf_sb[:], flagF[:, :])
            flagi_sb = const.tile([128, 1], F32)
            nc.sync.dma_start(flagi_sb[:], flagI[:, :])
            ones_sb = const.tile([128, 1], F32)
            nc.vector.memset(ones_sb[:], 1.0)

            masks = (mA_sb, mB_sb)

            for b in range(B):
                # ---- QKV projection for batch b ----
                qT = qkvp.tile([128, 2, T], F32R, tag="qT")
                kT = qkvp.tile([128, 2, T], F32R, tag="kT")
                vat = [
                    qkvp.tile([128, TK, 128], BF16, tag=f"v{s}", name=f"vat{s}")
                    for s in (0, 1)
                ]
                for tw in range(NW):
                    xw = xwp.tile([128, CK, W], BF16, tag="xw")
                    nc.sync.dma_start(
                        xw[:],
                        xT[b].rearrange("(ck p) t -> p ck t", p=128)[
                            :, :, tw * W : (tw + 1) * W
                        ],
                    )
                    # q/k feature-major: out [f 128, t W]
                    for s in range(4):  # qA qB kA kB
                        acc = ps_a.tile([128, W], F32, tag="mm512")
                        for ck in range(CK):
                            nc.tensor.matmul(
                                acc[:],
                                wq_sb[:, ck, s, :],
                                xw[:, ck, :],
                                start=(ck == 0),
                                stop=(ck == CK - 1),
                            )
                        dst = qT if s < 2 else kT
                        nc.vector.tensor_copy(
                            dst[:, s % 2, tw * W : (tw + 1) * W], acc[:]
                        )
                    # v token-major: out [t 128, (slot, d) 256]
                    for sub in range(W // 128):
                        ckg = tw * (W // 128) + sub
                        accv = ps_v.tile([128, 2, 128], F32, tag="vmm")
                        for ck in range(CK):
                            nc.tensor.matmul(
                                accv[:],
                                xw[:, ck, sub * 128 : (sub + 1) * 128],
                                wq_sb[:, ck, 4:6, :],
                                start=(ck == 0),
                                stop=(ck == CK - 1),
                            )
                        vstage = ywp.tile([128, 2, 128], F32, tag="ystage")
                        for s in (0, 1):
                            nc.vector.tensor_copy(
                                vat[s][:, ckg, :], accv[:, s, :]
                            )
                            nc.vector.tensor_copy(vstage[:, s, :], accv[:, s, :])
                            nc.sync.dma_start(
                                curv[b, s, ckg * 128 : (ckg + 1) * 128, :],
                                vstage[:, s, :],
                            )

                # cur_k out (feature-major; host transposes)
                for s in (0, 1):
                    nc.sync.dma_start(curk[b, s], kT[:, s, :])

                # ---- slot-B attention k/v select (self proj vs prev cache) ----
                pk_sb = prevp.tile([128, T], BF16, tag="pk")
                nc.sync.dma_start(pk_sb[:], prevKT[b])
                # kT_B = kT_B * (1-flag) + prev * flag   (fp32r-safe blend)
                nc.vector.tensor_scalar_mul(kT[:, 1, :], kT[:, 1, :], flagi_sb[:])
                nc.vector.tensor_scalar_mul(pk_sb[:], pk_sb[:], flagf_sb[:])
                nc.vector.tensor_tensor(
                    kT[:, 1, :], kT[:, 1, :], pk_sb[:], mybir.AluOpType.add
                )
                pv_sb = prevp.tile([128, TK, 128], BF16, tag="pv")
                nc.sync.dma_start(
                    pv_sb[:], prevV[b].rearrange("(ck p) d -> p ck d", p=128)
                )
                nc.vector.copy_predicated(
                    vat[1][:], flag_sb[:].to_broadcast((128, TK, 128)), pv_sb[:]
                )

                # ---- attention + projection, per window ----
                yT = qkvp.tile([128, 2, T], BF16, tag="yT")
                for wp2 in range(NW // 2):
                  for w in (2 * wp2, 2 * wp2 + 1):
                    yTw = yT[:, :, w * W : (w + 1) * W]
                    for slot in (0, 1):
                        nch = (4 * w + 4) if slot == 0 else TK
                        pT = ptp.tile([128, TK, W], BF16, tag="pT")
                        S = attnp.tile([128, W], F32, tag="S")
                        for j in range(nch):
                            accs = ps_a.tile([128, W], F32, tag="mm512")
                            nc.tensor.matmul(
                                accs[:],
                                kT[:, slot, j * 128 : (j + 1) * 128],
                                qT[:, slot, w * W : (w + 1) * W],
                                start=True,
                                stop=True,
                            )
                            nc.scalar.activation(pT[:, j, :], accs[:], AF.Exp)
                            r = min(max(j - 4 * w, -1), 4)
                            if r >= 0:
                                off = 512 - 128 * r
                                nc.vector.tensor_tensor(
                                    pT[:, j, :],
                                    pT[:, j, :],
                                    masks[slot][:, off : off + W],
                                    mybir.AluOpType.mult,
                                )
                            if j == 0:
                                nc.vector.tensor_copy(S[:], pT[:, 0, :])
                            else:
                                nc.vector.tensor_tensor(
                                    S[:], S[:], pT[:, j, :], mybir.AluOpType.add
                                )
                        # denominator: column sums of S via one ones-matmul
                        dps = ps_d.tile([1, W], F32, tag="den")
                        nc.tensor.matmul(
                            dps[:], ones_sb[:], S[:], start=True, stop=True
                        )
                        recip = attnp.tile([1, W], F32, tag="recip")
                        nc.vector.reciprocal(recip[:], dps[:])
                        recipb = attnp.tile([128, W], F32, tag="recipb")
                        nc.gpsimd.partition_broadcast(recipb[:], recip[:])
                        # PV: yT [d 128, q W] accumulated over k chunks
                        accy = ps_y.tile([128, W], F32, tag="pv512")
                        for j in range(nch):
                            nc.tensor.matmul(
                                accy[:],
                                vat[slot][:, j, :],
                                pT[:, j, :],
                                start=(j == 0),
                                stop=(j == nch - 1),
                            )
                        nc.vector.tensor_tensor(
                            yTw[:, slot, :], accy[:], recipb[:], mybir.AluOpType.mult
                        )

                  # output projection for this window pair, feature-major out
                  for oc in range(C // 128):
                        accps = [
                            ps_a.tile([128, W], F32, tag="mm512", name=f"accp{i}")
                            for i in range(2)
                        ]
                        for slot in (0, 1):
                            for i in range(2):
                                w = wp2 * 2 + i
                                nc.tensor.matmul(
                                    accps[i][:],
                                    wp_sb[:, slot, oc * 128 : (oc + 1) * 128],
                                    yT[:, slot, w * W : (w + 1) * W],
                                    start=(slot == 0),
                                    stop=(slot == 1),
                                )
                        for i in range(2):
                            w = wp2 * 2 + i
                            ystage = ywp.tile([128, W], F32, tag="ystage")
                            nc.vector.tensor_copy(ystage[:], accps[i][:])
                            nc.sync.dma_start(
                                y_out[b, oc * 128 : (oc + 1) * 128,
                                      w * W : (w + 1) * W],
                                ystage[:],
                            )
    nc.compile()
    return nc


_CACHE = {}


def _get_nc():
    if "nc" not in _CACHE:
        nc = bacc.Bacc(None, target_bir_lowering=False, debug=False)
        _CACHE["nc"] = _emit(nc)
    return _CACHE["nc"]


def _host_inputs(x, w_attn, w_proj, prevs):
    """Build the 8 per-core input maps."""
    xTh = np.ascontiguousarray(x.transpose(0, 2, 1)).astype(ml_dtypes.bfloat16)
    scale = np.float32(1.0 / np.sqrt(D))

    kk = np.arange(128)[:, None]
    cc = np.arange(MCOLS)[None, :]
    stair = (cc >= kk + 512).astype(ml_dtypes.bfloat16)
    ones_m = np.ones((128, MCOLS), dtype=ml_dtypes.bfloat16)
    zkt = np.zeros((B, 128, T), dtype=ml_dtypes.bfloat16)
    zv = np.zeros((B, T, 128), dtype=ml_dtypes.bfloat16)

    in_maps = []
    for c in range(N_CORES):
        hA, hB = _core_heads(c)
        cols = []
        for h in (hA, hB):
            cols.append(w_attn[h * D : (h + 1) * D, :].T * scale)  # q (scaled)
        for h in (hA, hB):
            cols.append(w_attn[C + h * D : C + (h + 1) * D, :].T)  # k
        for h in (hA, hB):
            cols.append(w_attn[2 * C + h * D : 2 * C + (h + 1) * D, :].T)  # v
        # order: qA qB kA kB vA vB -> [C, 6, 128]
        wq = np.stack(cols, axis=1).astype(ml_dtypes.bfloat16)
        wp = np.stack(
            [w_proj[:, hA * D : (hA + 1) * D].T, w_proj[:, hB * D : (hB + 1) * D].T],
            axis=0,
        ).astype(ml_dtypes.bfloat16)  # [2, 128, C]

        cross = c >= 4
        if cross:
            i = hB - SA
            pk, pv = prevs[i]
            pkt = np.ascontiguousarray(pk[:, hB].transpose(0, 2, 1)).astype(
                ml_dtypes.bfloat16
            )  # [B,128,T]
            pvv = np.ascontiguousarray(pv[:, hB]).astype(ml_dtypes.bfloat16)
        else:
            pkt, pvv = zkt, zv

        in_maps.append(
            {
                "xT": xTh,
                "wqkv": np.ascontiguousarray(wq),
                "wproj": np.ascontiguousarray(wp),
                "prevKT": pkt,
                "prevV": pvv,
                "maskA": stair,
                "maskB": ones_m if cross else stair,
                "flagB": np.full((128, 1), 1 if cross else 0, dtype=np.uint8),
                "flagF": np.full((128, 1), 1.0 if cross else 0.0, dtype=np.float32),
                "onesP": np.ones((128, 1), dtype=np.float32),
                "flagI": np.full((128, 1), 0.0 if cross else 1.0, dtype=np.float32),
            }
        )
    return in_maps


def kernel(x, w_attn, w_proj,
           prev_k0, prev_v0, prev_k1, prev_v1,
           prev_k2, prev_v2, prev_k3, prev_v3,
           _trace=False):
    x = np.asarray(x, dtype=np.float32)
    w_attn = np.asarray(w_attn, dtype=np.float32)
    w_proj = np.asarray(w_proj, dtype=np.float32)
    prevs = [
        (np.asarray(prev_k0), np.asarray(prev_v0)),
        (np.asarray(prev_k1), np.asarray(prev_v1)),
        (np.asarray(prev_k2), np.asarray(prev_v2)),
        (np.asarray(prev_k3), np.asarray(prev_v3)),
    ]
    nc = _get_nc()
    in_maps = _host_inputs(x, w_attn, w_proj, prevs)
    res = run_bass_kernel_spmd(
        nc, in_maps, core_ids=list(range(N_CORES)), trace=_trace
    )
    kernel.last_exec_time_ns = res.exec_time_ns

    y = np.zeros((B, C, T), dtype=np.float64)
    cur_k = np.zeros((B, H, T, D), dtype=np.float32)
    cur_v = np.zeros((B, H, T, D), dtype=np.float32)
    for c in range(N_CORES):
        out = res.results[c]
        y += out["y_out"].astype(np.float64)
        hA, hB = _core_heads(c)
        for s, h in ((0, hA), (1, hB)):
            cur_k[:, h] = out["curk"][:, s].transpose(0, 2, 1)
            cur_v[:, h] = out["curv"][:, s]
    return np.ascontiguousarray(y.transpose(0, 2, 1)).astype(np.float32), cur_k, cur_v


kernel.last_exec_time_ns = None


# revision 20
# speedup vs baseline: 1.3552x; 1.0071x over previous
"""TRN2 Bass kernel for nn_Attention_79628693668242 (sparse attention).

Head-parallel (tensor parallel) across 8 NeuronCores, 2 heads per core:
  cores 0-3: heads (2c, 2c+1)            -- both causal self-attention
  cores 4-7: heads (c+4, c+8)            -- one self head + one cross head

Uniform SPMD program; per-core behavior differs only through data:
  - mask tensors (causal staircase vs all-ones)
  - a flag that predicate-selects slot-B attention k/v between the current
    projection (self) and the prev-layer cache (cross)

All matmul operands are kept in layouts that need no transposes:
  - x is host-transposed to xT [B, C, T] (fp32r)
  - q,k projected feature-major [d, t]; v projected token-major [t, d]
  - PV computes yT = v.T @ pT feature-major, which directly feeds the
    output projection as lhsT; proj emits token-major y tiles.
Softmax skips the max-subtraction (scores are bounded ~|6| here, exp is
safe in fp32) which makes the whole softmax free-dim-local except for the
denominator: S = sum_chunks exp-chunks (DVE), column sums via a ones
matmul, reciprocal, gpsimd partition-broadcast, folded into the PV-psum
eviction multiply.
"""

import numpy as np
import ml_dtypes

import concourse.bass as bass
import concourse.tile as tile
from concourse import bacc, mybir
from concourse.bass_utils import run_bass_kernel_spmd

F32 = mybir.dt.float32
F32R = mybir.dt.float32r
BF16 = mybir.dt.bfloat16
AF = mybir.ActivationFunctionType

B, T, C, D = 4, 2048, 2048, 128
H, SA = 16, 12
CK = C // 128          # 16 contraction chunks
TK = T // 128          # 16 token chunks per batch
W = 512                # q / t window
NW = T // W            # 4 windows per batch
MCOLS = 1152           # mask columns: slice offsets 0..640

N_CORES = 8


def _core_heads(c):
    if c < 4:
        return 2 * c, 2 * c + 1
    return c + 4, c + 8


def _emit(nc):
    xT = nc.declare_dram_parameter("xT", [B, C, T], BF16, isOutput=False)
    wqkv = nc.declare_dram_parameter("wqkv", [C, 6, 128], BF16, isOutput=False)
    wproj = nc.declare_dram_parameter("wproj", [2, 128, C], BF16, isOutput=False)
    prevKT = nc.declare_dram_parameter("prevKT", [B, 128, T], BF16, isOutput=False)
    prevV = nc.declare_dram_parameter("prevV", [B, T, 128], BF16, isOutput=False)
    maskA = nc.declare_dram_parameter("maskA", [128, MCOLS], BF16, isOutput=False)
    maskB = nc.declare_dram_parameter("maskB", [128, MCOLS], BF16, isOutput=False)
    flagB = nc.declare_dram_parameter("flagB", [128, 1], mybir.dt.uint8, isOutput=False)
    flagF = nc.declare_dram_parameter("flagF", [128, 1], F32, isOutput=False)
    onesP = nc.declare_dram_parameter("onesP", [128, 1], F32R, isOutput=False)
    flagI = nc.declare_dram_parameter("flagI", [128, 1], F32, isOutput=False)

    y_out = nc.declare_dram_parameter("y_out", [B, C, T], F32, isOutput=True)
    curk = nc.declare_dram_parameter("curk", [B, 2, 128, T], F32R, isOutput=True)
    curv = nc.declare_dram_parameter("curv", [B, 2, T, 128], F32, isOutput=True)

    with tile.TileContext(nc) as tc:
        with (
            tc.tile_pool(name="const", bufs=1) as const,
            tc.tile_pool(name="xwp", bufs=2) as xwp,
            tc.tile_pool(name="qkvp", bufs=2) as qkvp,
            tc.tile_pool(name="attn", bufs=1) as attnp,
            tc.tile_pool(name="ptp", bufs=2) as ptp,
            tc.tile_pool(name="yw", bufs=2) as ywp,
            tc.tile_pool(name="prevp", bufs=1) as prevp,
            tc.tile_pool(name="ytp", bufs=1) as ytp,
            tc.tile_pool(name="ps_a", bufs=3, space="PSUM") as ps_a,
            tc.tile_pool(name="ps_y", bufs=2, space="PSUM") as ps_y,
            tc.tile_pool(name="ps_v", bufs=2, space="PSUM") as ps_v,
            tc.tile_pool(name="ps_d", bufs=1, space="PSUM") as ps_d,
        ):
            # ---- persistent loads ----
            wq_sb = const.tile([128, CK, 6, 128], BF16)
            nc.sync.dma_start(
                wq_sb[:], wqkv.rearrange("(ck p) s d -> p ck s d", p=128)
            )
            wp_sb = const.tile([128, 2, C], BF16)
            nc.sync.dma_start(wp_sb[:], wproj.rearrange("h p o -> p h o"))
            mA_sb = const.tile([128, MCOLS], BF16)
            nc.sync.dma_start(mA_sb[:], maskA[:, :])
            mB_sb = const.tile([128, MCOLS], BF16)
            nc.sync.dma_start(mB_sb[:], maskB[:, :])
            flag_sb = const.tile([128, 1], mybir.dt.uint8)
            nc.sync.dma_start(flag_sb[:], flagB[:, :])
            flagf_sb = const.tile([128, 1], F32)
            nc.sync.dma_start(flagf_sb[:], flagF[:, :])
            flagi_sb = const.tile([128, 1], F32)
            nc.sync.dma_start(flagi_sb[:], flagI[:, :])
            ones_sb = const.tile([128, 1], F32)
            nc.vector.memset(ones_sb[:], 1.0)

            masks = (mA_sb, mB_sb)

            for b in range(B):
                # ---- QKV projection for batch b ----
                qT = qkvp.tile([128, 2, T], F32R, tag="qT")
                kT = qkvp.tile([128, 2, T], F32R, tag="kT")
                vat = [
                    qkvp.tile([128, TK, 128], BF16, tag=f"v{s}", name=f"vat{s}")
                    for s in (0, 1)
                ]
                for tw in range(NW):
                    xw = xwp.tile([128, CK, W], BF16, tag="xw")
                    nc.sync.dma_start(
                        xw[:],
                        xT[b].rearrange("(ck p) t -> p ck t", p=128)[
                            :, :, tw * W : (tw + 1) * W
                        ],
                    )
                    # q/k feature-major: out [f 128, t W]
                    for s in range(4):  # qA qB kA kB
                        acc = ps_a.tile([128, W], F32, tag="mm512")
                        for ck in range(CK):
                            nc.tensor.matmul(
                                acc[:],
                                wq_sb[:, ck, s, :],
                                xw[:, ck, :],
                                start=(ck == 0),
                                stop=(ck == CK - 1),
                            )
                        dst = qT if s < 2 else kT
                        nc.vector.tensor_copy(
                            dst[:, s % 2, tw * W : (tw + 1) * W], acc[:]
                        )
                    # v token-major: out [t 128, (slot, d) 256]
                    for sub in range(W // 128):
                        ckg = tw * (W // 128) + sub
                        accv = ps_v.tile([128, 2, 128], F32, tag="vmm")
                        for ck in range(CK):
                            nc.tensor.matmul(
                                accv[:],
                                xw[:, ck, sub * 128 : (sub + 1) * 128],
                                wq_sb[:, ck, 4:6, :],
                                start=(ck == 0),
                                stop=(ck == CK - 1),
                            )
                        vstage = ywp.tile([128, 2, 128], F32, tag="ystage")
                        for s in (0, 1):
                            nc.vector.tensor_copy(
                                vat[s][:, ckg, :], accv[:, s, :]
                            )
                            nc.vector.tensor_copy(vstage[:, s, :], accv[:, s, :])
                            nc.sync.dma_start(
                                curv[b, s, ckg * 128 : (ckg + 1) * 128, :],
                                vstage[:, s, :],
                            )

                # cur_k out (feature-major; host transposes)
                for s in (0, 1):
                    nc.sync.dma_start(curk[b, s], kT[:, s, :])

                # ---- slot-B attention k/v select (self proj vs prev cache) ----
                pk_sb = prevp.tile([128, T], BF16, tag="pk")
                nc.sync.dma_start(pk_sb[:], prevKT[b])
                # kT_B = kT_B * (1-flag) + prev * flag   (fp32r-safe blend)
                nc.vector.tensor_scalar_mul(kT[:, 1, :], kT[:, 1, :], flagi_sb[:])
                nc.vector.tensor_scalar_mul(pk_sb[:], pk_sb[:], flagf_sb[:])
                nc.vector.tensor_tensor(
                    kT[:, 1, :], kT[:, 1, :], pk_sb[:], mybir.AluOpType.add
                )
                pv_sb = prevp.tile([128, TK, 128], BF16, tag="pv")
                nc.sync.dma_start(
                    pv_sb[:], prevV[b].rearrange("(ck p) d -> p ck d", p=128)
                )
                nc.vector.copy_predicated(
                    vat[1][:], flag_sb[:].to_broadcast((128, TK, 128)), pv_sb[:]
                )

                # ---- attention + projection, per window ----
                yT = ytp.tile([128, 2, T], BF16, tag="yT")
                for wp2 in range(NW // 2):
                  for w in (2 * wp2, 2 * wp2 + 1):
                    yTw = yT[:, :, w * W : (w + 1) * W]
                    for slot in (0, 1):
                        nch = (4 * w + 4) if slot == 0 else TK
                        pT = ptp.tile([128, TK, W], BF16, tag="pT")
                        S = attnp.tile([128, W], F32, tag="S")
                        for j in range(nch):
                            accs = ps_a.tile([128, W], F32, tag="mm512")
                            nc.tensor.matmul(
                                accs[:],
                                kT[:, slot, j * 128 : (j + 1) * 128],
                                qT[:, slot, w * W : (w + 1) * W],
                                start=True,
                                stop=True,
                            )
                            nc.scalar.activation(pT[:, j, :], accs[:], AF.Exp)
                            r = min(max(j - 4 * w, -1), 4)
                            if r >= 0:
                                off = 512 - 128 * r
                                nc.vector.tensor_tensor(
                                    pT[:, j, :],
                                    pT[:, j, :],
                                    masks[slot][:, off : off + W],
                                    mybir.AluOpType.mult,
                                )
                            if j == 0:
                                nc.vector.tensor_copy(S[:], pT[:, 0, :])
                            else:
                                nc.vector.tensor_tensor(
                                    S[:], S[:], pT[:, j, :], mybir.AluOpType.add
                                )
                        # PV: yT [d 128, q W] accumulated over k chunks
                        accy = ps_y.tile([128, W], F32, tag="pv512")
                        for j in range(nch):
                            nc.tensor.matmul(
                                accy[:],
                                vat[slot][:, j, :],
                                pT[:, j, :],
                                start=(j == 0),
                                stop=(j == nch - 1),
                            )
                        # denominator: column sums of S via one ones-matmul
                        dps = ps_d.tile([1, W], F32, tag="den")
                        nc.tensor.matmul(
                            dps[:], ones_sb[:], S[:], start=True, stop=True
                        )
                        recip = attnp.tile([1, W], F32, tag="recip")
                        nc.vector.reciprocal(recip[:], dps[:])
                        recipb = attnp.tile([128, W], F32, tag="recipb")
                        nc.gpsimd.partition_broadcast(recipb[:], recip[:])
                        nc.vector.tensor_tensor(
                            yTw[:, slot, :], accy[:], recipb[:], mybir.AluOpType.mult
                        )

                  # output projection for this window pair, feature-major out
                  for oc in range(C // 128):
                        accps = [
                            ps_a.tile([128, W], F32, tag="mm512", name=f"accp{i}")
                            for i in range(2)
                        ]
                        for slot in (0, 1):
                            for i in range(2):
                                w = wp2 * 2 + i
                                nc.tensor.matmul(
                                    accps[i][:],
                                    wp_sb[:, slot, oc * 128 : (oc + 1) * 128],
                                    yT[:, slot, w * W : (w + 1) * W],
                                    start=(slot == 0),
                                    stop=(slot == 1),
                                )
                        for i in range(2):
                            w = wp2 * 2 + i
                            ystage = ywp.tile([128, W], F32, tag="ystage")
                            nc.vector.tensor_copy(ystage[:], accps[i][:])
                            nc.sync.dma_start(
                                y_out[b, oc * 128 : (oc + 1) * 128,
                                      w * W : (w + 1) * W],
                                ystage[:],
                            )
    nc.compile()
    return nc


_CACHE = {}


def _get_nc():
    if "nc" not in _CACHE:
        nc = bacc.Bacc(None, target_bir_lowering=False, debug=False)
        _CACHE["nc"] = _emit(nc)
    return _CACHE["nc"]


def _host_inputs(x, w_attn, w_proj, prevs):
    """Build the 8 per-core input maps."""
    xTh = np.ascontiguousarray(x.transpose(0, 2, 1)).astype(ml_dtypes.bfloat16)
    scale = np.float32(1.0 / np.sqrt(D))

    kk = np.arange(128)[:, None]
    cc = np.arange(MCOLS)[None, :]
    stair = (cc >= kk + 512).astype(ml_dtypes.bfloat16)
    ones_m = np.ones((128, MCOLS), dtype=ml_dtypes.bfloat16)
    zkt = np.zeros((B, 128, T), dtype=ml_dtypes.bfloat16)
    zv = np.zeros((B, T, 128), dtype=ml_dtypes.bfloat16)

    in_maps = []
    for c in range(N_CORES):
        hA, hB = _core_heads(c)
        cols = []
        for h in (hA, hB):
            cols.append(w_attn[h * D : (h + 1) * D, :].T * scale)  # q (scaled)
        for h in (hA, hB):
            cols.append(w_attn[C + h * D : C + (h + 1) * D, :].T)  # k
        for h in (hA, hB):
            cols.append(w_attn[2 * C + h * D : 2 * C + (h + 1) * D, :].T)  # v
        # order: qA qB kA kB vA vB -> [C, 6, 128]
        wq = np.stack(cols, axis=1).astype(ml_dtypes.bfloat16)
        wp = np.stack(
            [w_proj[:, hA * D : (hA + 1) * D].T, w_proj[:, hB * D : (hB + 1) * D].T],
            axis=0,
        ).astype(ml_dtypes.bfloat16)  # [2, 128, C]

        cross = c >= 4
        if cross:
            i = hB - SA
            pk, pv = prevs[i]
            pkt = np.ascontiguousarray(pk[:, hB].transpose(0, 2, 1)).astype(
                ml_dtypes.bfloat16
            )  # [B,128,T]
            pvv = np.ascontiguousarray(pv[:, hB]).astype(ml_dtypes.bfloat16)
        else:
            pkt, pvv = zkt, zv

        in_maps.append(
            {
                "xT": xTh,
                "wqkv": np.ascontiguousarray(wq),
                "wproj": np.ascontiguousarray(wp),
                "prevKT": pkt,
                "prevV": pvv,
                "maskA": stair,
                "maskB": ones_m if cross else stair,
                "flagB": np.full((128, 1), 1 if cross else 0, dtype=np.uint8),
                "flagF": np.full((128, 1), 1.0 if cross else 0.0, dtype=np.float32),
                "onesP": np.ones((128, 1), dtype=np.float32),
                "flagI": np.full((128, 1), 0.0 if cross else 1.0, dtype=np.float32),
            }
        )
    return in_maps


def kernel(x, w_attn, w_proj,
           prev_k0, prev_v0, prev_k1, prev_v1,
           prev_k2, prev_v2, prev_k3, prev_v3,
           _trace=False):
    x = np.asarray(x, dtype=np.float32)
    w_attn = np.asarray(w_attn, dtype=np.float32)
    w_proj = np.asarray(w_proj, dtype=np.float32)
    prevs = [
        (np.asarray(prev_k0), np.asarray(prev_v0)),
        (np.asarray(prev_k1), np.asarray(prev_v1)),
        (np.asarray(prev_k2), np.asarray(prev_v2)),
        (np.asarray(prev_k3), np.asarray(prev_v3)),
    ]
    nc = _get_nc()
    in_maps = _host_inputs(x, w_attn, w_proj, prevs)
    res = run_bass_kernel_spmd(
        nc, in_maps, core_ids=list(range(N_CORES)), trace=_trace
    )
    kernel.last_exec_time_ns = res.exec_time_ns

    y = np.zeros((B, C, T), dtype=np.float64)
    cur_k = np.zeros((B, H, T, D), dtype=np.float32)
    cur_v = np.zeros((B, H, T, D), dtype=np.float32)
    for c in range(N_CORES):
        out = res.results[c]
        y += out["y_out"].astype(np.float64)
        hA, hB = _core_heads(c)
        for s, h in ((0, hA), (1, hB)):
            cur_k[:, h] = out["curk"][:, s].transpose(0, 2, 1)
            cur_v[:, h] = out["curv"][:, s]
    return np.ascontiguousarray(y.transpose(0, 2, 1)).astype(np.float32), cur_k, cur_v


kernel.last_exec_time_ns = None


# revision 21
# speedup vs baseline: 1.5325x; 1.1309x over previous
"""TRN2 Bass kernel for nn_Attention_79628693668242 (sparse attention).

Head-parallel (tensor parallel) across 8 NeuronCores, 2 heads per core:
  cores 0-3: heads (2c, 2c+1)            -- both causal self-attention
  cores 4-7: heads (c+4, c+8)            -- one self head + one cross head

Uniform SPMD program; per-core behavior differs only through data:
  - mask tensors (causal staircase vs all-ones)
  - a flag that predicate-selects slot-B attention k/v between the current
    projection (self) and the prev-layer cache (cross)

All matmul operands are kept in layouts that need no transposes:
  - x is host-transposed to xT [B, C, T] (fp32r)
  - q,k projected feature-major [d, t]; v projected token-major [t, d]
  - PV computes yT = v.T @ pT feature-major, which directly feeds the
    output projection as lhsT; proj emits token-major y tiles.
Softmax skips the max-subtraction (scores are bounded ~|6| here, exp is
safe in fp32) which makes the whole softmax free-dim-local except for the
denominator: S = sum_chunks exp-chunks (DVE), column sums via a ones
matmul, reciprocal, gpsimd partition-broadcast, folded into the PV-psum
eviction multiply.
"""

import numpy as np
import ml_dtypes

import concourse.bass as bass
import concourse.tile as tile
from concourse import bacc, mybir
from concourse.bass_utils import run_bass_kernel_spmd

F32 = mybir.dt.float32
F32R = mybir.dt.float32r
BF16 = mybir.dt.bfloat16
AF = mybir.ActivationFunctionType

B, T, C, D = 4, 2048, 2048, 128
H, SA = 16, 12
CK = C // 128          # 16 contraction chunks
TK = T // 128          # 16 token chunks per batch
W = 512                # q / t window
NW = T // W            # 4 windows per batch
MCOLS = 1152           # mask columns: slice offsets 0..640

N_CORES = 8


def _core_heads(c):
    if c < 4:
        return 2 * c, 2 * c + 1
    return c + 4, c + 8


def _emit(nc):
    xT = nc.declare_dram_parameter("xT", [B, C, T], BF16, isOutput=False)
    wqkv = nc.declare_dram_parameter("wqkv", [C, 6, 128], BF16, isOutput=False)
    wproj = nc.declare_dram_parameter("wproj", [2, 128, C], BF16, isOutput=False)
    prevKT = nc.declare_dram_parameter("prevKT", [B, 128, T], BF16, isOutput=False)
    prevV = nc.declare_dram_parameter("prevV", [B, T, 128], BF16, isOutput=False)
    maskA = nc.declare_dram_parameter("maskA", [128, MCOLS], BF16, isOutput=False)
    maskB = nc.declare_dram_parameter("maskB", [128, MCOLS], BF16, isOutput=False)
    flagB = nc.declare_dram_parameter("flagB", [128, 1], mybir.dt.uint8, isOutput=False)
    flagF = nc.declare_dram_parameter("flagF", [128, 1], F32, isOutput=False)
    onesP = nc.declare_dram_parameter("onesP", [128, 1], F32R, isOutput=False)
    flagI = nc.declare_dram_parameter("flagI", [128, 1], F32, isOutput=False)

    y_out = nc.declare_dram_parameter("y_out", [B, C, T], F32, isOutput=True)
    curk = nc.declare_dram_parameter("curk", [B, 2, 128, T], F32R, isOutput=True)
    curv = nc.declare_dram_parameter("curv", [B, 2, T, 128], F32, isOutput=True)

    with tile.TileContext(nc) as tc:
        with (
            tc.tile_pool(name="const", bufs=1) as const,
            tc.tile_pool(name="xwp", bufs=2) as xwp,
            tc.tile_pool(name="qkvp", bufs=2) as qkvp,
            tc.tile_pool(name="attn", bufs=1) as attnp,
            tc.tile_pool(name="ptp", bufs=2) as ptp,
            tc.tile_pool(name="yw", bufs=2) as ywp,
            tc.tile_pool(name="prevp", bufs=1) as prevp,
            tc.tile_pool(name="ytp", bufs=1) as ytp,
            tc.tile_pool(name="ps_a", bufs=3, space="PSUM") as ps_a,
            tc.tile_pool(name="ps_y", bufs=2, space="PSUM") as ps_y,
            tc.tile_pool(name="ps_v", bufs=2, space="PSUM") as ps_v,
            tc.tile_pool(name="ps_d", bufs=1, space="PSUM") as ps_d,
        ):
            # ---- persistent loads ----
            wq_sb = const.tile([128, CK, 6, 128], BF16)
            nc.sync.dma_start(
                wq_sb[:], wqkv.rearrange("(ck p) s d -> p ck s d", p=128)
            )
            wp_sb = const.tile([128, 2, C], BF16)
            nc.sync.dma_start(wp_sb[:], wproj.rearrange("h p o -> p h o"))
            mA_sb = const.tile([128, MCOLS], BF16)
            nc.sync.dma_start(mA_sb[:], maskA[:, :])
            mB_sb = const.tile([128, MCOLS], BF16)
            nc.sync.dma_start(mB_sb[:], maskB[:, :])
            flag_sb = const.tile([128, 1], mybir.dt.uint8)
            nc.sync.dma_start(flag_sb[:], flagB[:, :])
            flagf_sb = const.tile([128, 1], F32)
            nc.sync.dma_start(flagf_sb[:], flagF[:, :])
            flagi_sb = const.tile([128, 1], F32)
            nc.sync.dma_start(flagi_sb[:], flagI[:, :])
            onesM = const.tile([128, 128], BF16)
            nc.vector.memset(onesM[:], 1.0)

            masks = (mA_sb, mB_sb)

            for b in range(B):
                # ---- QKV projection for batch b ----
                qT = qkvp.tile([128, 2, T], F32R, tag="qT")
                kT = qkvp.tile([128, 2, T], F32R, tag="kT")
                vat = [
                    qkvp.tile([128, TK, 128], BF16, tag=f"v{s}", name=f"vat{s}")
                    for s in (0, 1)
                ]
                for tw in range(NW):
                    xw = xwp.tile([128, CK, W], BF16, tag="xw")
                    nc.sync.dma_start(
                        xw[:],
                        xT[b].rearrange("(ck p) t -> p ck t", p=128)[
                            :, :, tw * W : (tw + 1) * W
                        ],
                    )
                    # q/k feature-major: out [f 128, t W]
                    for s in range(4):  # qA qB kA kB
                        acc = ps_a.tile([128, W], F32, tag="mm512")
                        for ck in range(CK):
                            nc.tensor.matmul(
                                acc[:],
                                wq_sb[:, ck, s, :],
                                xw[:, ck, :],
                                start=(ck == 0),
                                stop=(ck == CK - 1),
                            )
                        dst = qT if s < 2 else kT
                        nc.vector.tensor_copy(
                            dst[:, s % 2, tw * W : (tw + 1) * W], acc[:]
                        )
                    # v token-major: out [t 128, (slot, d) 256]
                    for sub in range(W // 128):
                        ckg = tw * (W // 128) + sub
                        accv = ps_v.tile([128, 2, 128], F32, tag="vmm")
                        for ck in range(CK):
                            nc.tensor.matmul(
                                accv[:],
                                xw[:, ck, sub * 128 : (sub + 1) * 128],
                                wq_sb[:, ck, 4:6, :],
                                start=(ck == 0),
                                stop=(ck == CK - 1),
                            )
                        vstage = ywp.tile([128, 2, 128], F32, tag="ystage")
                        for s in (0, 1):
                            nc.vector.tensor_copy(
                                vat[s][:, ckg, :], accv[:, s, :]
                            )
                            nc.vector.tensor_copy(vstage[:, s, :], accv[:, s, :])
                            nc.sync.dma_start(
                                curv[b, s, ckg * 128 : (ckg + 1) * 128, :],
                                vstage[:, s, :],
                            )

                # cur_k out (feature-major; host transposes)
                for s in (0, 1):
                    nc.sync.dma_start(curk[b, s], kT[:, s, :])

                # ---- slot-B attention k/v select (self proj vs prev cache) ----
                pk_sb = prevp.tile([128, T], BF16, tag="pk")
                nc.sync.dma_start(pk_sb[:], prevKT[b])
                # kT_B = kT_B * (1-flag) + prev * flag   (fp32r-safe blend)
                nc.vector.tensor_scalar_mul(kT[:, 1, :], kT[:, 1, :], flagi_sb[:])
                nc.vector.tensor_scalar_mul(pk_sb[:], pk_sb[:], flagf_sb[:])
                nc.vector.tensor_tensor(
                    kT[:, 1, :], kT[:, 1, :], pk_sb[:], mybir.AluOpType.add
                )
                pv_sb = prevp.tile([128, TK, 128], BF16, tag="pv")
                nc.sync.dma_start(
                    pv_sb[:], prevV[b].rearrange("(ck p) d -> p ck d", p=128)
                )
                nc.vector.copy_predicated(
                    vat[1][:], flag_sb[:].to_broadcast((128, TK, 128)), pv_sb[:]
                )

                # ---- attention + projection, per window ----
                yT = ytp.tile([128, 2, T], BF16, tag="yT")
                for wp2 in range(NW // 2):
                  for w in (2 * wp2, 2 * wp2 + 1):
                    yTw = yT[:, :, w * W : (w + 1) * W]
                    for slot in (0, 1):
                        nch = (4 * w + 4) if slot == 0 else TK
                        pT = ptp.tile([128, TK, W], BF16, tag="pT")
                        for j in range(nch):
                            accs = ps_a.tile([128, W], F32, tag="mm512")
                            nc.tensor.matmul(
                                accs[:],
                                kT[:, slot, j * 128 : (j + 1) * 128],
                                qT[:, slot, w * W : (w + 1) * W],
                                start=True,
                                stop=True,
                            )
                            nc.scalar.activation(pT[:, j, :], accs[:], AF.Exp)
                            r = min(max(j - 4 * w, -1), 4)
                            if r >= 0:
                                off = 512 - 128 * r
                                nc.vector.tensor_tensor(
                                    pT[:, j, :],
                                    pT[:, j, :],
                                    masks[slot][:, off : off + W],
                                    mybir.AluOpType.mult,
                                )
                        # PV and denominator accumulated together:
                        # den rows are all equal to the masked-exp column sums
                        accy = ps_y.tile([128, W], F32, tag="pv512")
                        dend = ps_d.tile([128, W], F32, tag="den")
                        for j in range(nch):
                            nc.tensor.matmul(
                                dend[:],
                                onesM[:],
                                pT[:, j, :],
                                start=(j == 0),
                                stop=(j == nch - 1),
                            )
                            nc.tensor.matmul(
                                accy[:],
                                vat[slot][:, j, :],
                                pT[:, j, :],
                                start=(j == 0),
                                stop=(j == nch - 1),
                            )
                        recipb = attnp.tile([128, W], F32, tag="recipb")
                        nc.vector.reciprocal(recipb[:], dend[:])
                        nc.vector.tensor_tensor(
                            yTw[:, slot, :], accy[:], recipb[:], mybir.AluOpType.mult
                        )

                  # output projection for this window pair, feature-major out
                  for oc in range(C // 128):
                        accps = [
                            ps_a.tile([128, W], F32, tag="mm512", name=f"accp{i}")
                            for i in range(2)
                        ]
                        for slot in (0, 1):
                            for i in range(2):
                                w = wp2 * 2 + i
                                nc.tensor.matmul(
                                    accps[i][:],
                                    wp_sb[:, slot, oc * 128 : (oc + 1) * 128],
                                    yT[:, slot, w * W : (w + 1) * W],
                                    start=(slot == 0),
                                    stop=(slot == 1),
                                )
                        for i in range(2):
                            w = wp2 * 2 + i
                            ystage = ywp.tile([128, W], F32, tag="ystage")
                            nc.vector.tensor_copy(ystage[:], accps[i][:])
                            nc.sync.dma_start(
                                y_out[b, oc * 128 : (oc + 1) * 128,
                                      w * W : (w + 1) * W],
                                ystage[:],
                            )
    nc.compile()
    return nc


_CACHE = {}


def _get_nc():
    if "nc" not in _CACHE:
        nc = bacc.Bacc(None, target_bir_lowering=False, debug=False)
        _CACHE["nc"] = _emit(nc)
    return _CACHE["nc"]


def _host_inputs(x, w_attn, w_proj, prevs):
    """Build the 8 per-core input maps."""
    xTh = np.ascontiguousarray(x.transpose(0, 2, 1)).astype(ml_dtypes.bfloat16)
    scale = np.float32(1.0 / np.sqrt(D))

    kk = np.arange(128)[:, None]
    cc = np.arange(MCOLS)[None, :]
    stair = (cc >= kk + 512).astype(ml_dtypes.bfloat16)
    ones_m = np.ones((128, MCOLS), dtype=ml_dtypes.bfloat16)
    zkt = np.zeros((B, 128, T), dtype=ml_dtypes.bfloat16)
    zv = np.zeros((B, T, 128), dtype=ml_dtypes.bfloat16)

    in_maps = []
    for c in range(N_CORES):
        hA, hB = _core_heads(c)
        cols = []
        for h in (hA, hB):
            cols.append(w_attn[h * D : (h + 1) * D, :].T * scale)  # q (scaled)
        for h in (hA, hB):
            cols.append(w_attn[C + h * D : C + (h + 1) * D, :].T)  # k
        for h in (hA, hB):
            cols.append(w_attn[2 * C + h * D : 2 * C + (h + 1) * D, :].T)  # v
        # order: qA qB kA kB vA vB -> [C, 6, 128]
        wq = np.stack(cols, axis=1).astype(ml_dtypes.bfloat16)
        wp = np.stack(
            [w_proj[:, hA * D : (hA + 1) * D].T, w_proj[:, hB * D : (hB + 1) * D].T],
            axis=0,
        ).astype(ml_dtypes.bfloat16)  # [2, 128, C]

        cross = c >= 4
        if cross:
            i = hB - SA
            pk, pv = prevs[i]
            pkt = np.ascontiguousarray(pk[:, hB].transpose(0, 2, 1)).astype(
                ml_dtypes.bfloat16
            )  # [B,128,T]
            pvv = np.ascontiguousarray(pv[:, hB]).astype(ml_dtypes.bfloat16)
        else:
            pkt, pvv = zkt, zv

        in_maps.append(
            {
                "xT": xTh,
                "wqkv": np.ascontiguousarray(wq),
                "wproj": np.ascontiguousarray(wp),
                "prevKT": pkt,
                "prevV": pvv,
                "maskA": stair,
                "maskB": ones_m if cross else stair,
                "flagB": np.full((128, 1), 1 if cross else 0, dtype=np.uint8),
                "flagF": np.full((128, 1), 1.0 if cross else 0.0, dtype=np.float32),
                "onesP": np.ones((128, 1), dtype=np.float32),
                "flagI": np.full((128, 1), 0.0 if cross else 1.0, dtype=np.float32),
            }
        )
    return in_maps


def kernel(x, w_attn, w_proj,
           prev_k0, prev_v0, prev_k1, prev_v1,
           prev_k2, prev_v2, prev_k3, prev_v3,
           _trace=False):
    x = np.asarray(x, dtype=np.float32)
    w_attn = np.asarray(w_attn, dtype=np.float32)
    w_proj = np.asarray(w_proj, dtype=np.float32)
    prevs = [
        (np.asarray(prev_k0), np.asarray(prev_v0)),
        (np.asarray(prev_k1), np.asarray(prev_v1)),
        (np.asarray(prev_k2), np.asarray(prev_v2)),
        (np.asarray(prev_k3), np.asarray(prev_v3)),
    ]
    nc = _get_nc()
    in_maps = _host_inputs(x, w_attn, w_proj, prevs)
    res = run_bass_kernel_spmd(
        nc, in_maps, core_ids=list(range(N_CORES)), trace=_trace
    )
    kernel.last_exec_time_ns = res.exec_time_ns

    y = np.zeros((B, C, T), dtype=np.float64)
    cur_k = np.zeros((B, H, T, D), dtype=np.float32)
    cur_v = np.zeros((B, H, T, D), dtype=np.float32)
    for c in range(N_CORES):
        out = res.results[c]
        y += out["y_out"].astype(np.float64)
        hA, hB = _core_heads(c)
        for s, h in ((0, hA), (1, hB)):
            cur_k[:, h] = out["curk"][:, s].transpose(0, 2, 1)
            cur_v[:, h] = out["curv"][:, s]
    return np.ascontiguousarray(y.transpose(0, 2, 1)).astype(np.float32), cur_k, cur_v


kernel.last_exec_time_ns = None


# revision 22
# speedup vs baseline: 1.5752x; 1.0279x over previous
"""TRN2 Bass kernel for nn_Attention_79628693668242 (sparse attention).

Head-parallel (tensor parallel) across 8 NeuronCores, 2 heads per core:
  cores 0-3: heads (2c, 2c+1)            -- both causal self-attention
  cores 4-7: heads (c+4, c+8)            -- one self head + one cross head

Uniform SPMD program; per-core behavior differs only through data:
  - mask tensors (causal staircase vs all-ones)
  - a flag that predicate-selects slot-B attention k/v between the current
    projection (self) and the prev-layer cache (cross)

All matmul operands are kept in layouts that need no transposes:
  - x is host-transposed to xT [B, C, T] (fp32r)
  - q,k projected feature-major [d, t]; v projected token-major [t, d]
  - PV computes yT = v.T @ pT feature-major, which directly feeds the
    output projection as lhsT; proj emits token-major y tiles.
Softmax skips the max-subtraction (scores are bounded ~|6| here, exp is
safe in fp32) which makes the whole softmax free-dim-local except for the
denominator: S = sum_chunks exp-chunks (DVE), column sums via a ones
matmul, reciprocal, gpsimd partition-broadcast, folded into the PV-psum
eviction multiply.
"""

import numpy as np
import ml_dtypes

import concourse.bass as bass
import concourse.tile as tile
from concourse import bacc, mybir
from concourse.bass_utils import run_bass_kernel_spmd

F32 = mybir.dt.float32
F32R = mybir.dt.float32r
BF16 = mybir.dt.bfloat16
AF = mybir.ActivationFunctionType

B, T, C, D = 4, 2048, 2048, 128
H, SA = 16, 12
CK = C // 128          # 16 contraction chunks
TK = T // 128          # 16 token chunks per batch
W = 512                # q / t window
NW = T // W            # 4 windows per batch
MCOLS = 1152           # mask columns: slice offsets 0..640

N_CORES = 8


def _core_heads(c):
    if c < 4:
        return 2 * c, 2 * c + 1
    return c + 4, c + 8


def _emit(nc):
    xT = nc.declare_dram_parameter("xT", [B, C, T], BF16, isOutput=False)
    wqkv = nc.declare_dram_parameter("wqkv", [C, 6, 128], BF16, isOutput=False)
    wproj = nc.declare_dram_parameter("wproj", [2, 128, C], BF16, isOutput=False)
    prevKT = nc.declare_dram_parameter("prevKT", [B, 128, T], BF16, isOutput=False)
    prevV = nc.declare_dram_parameter("prevV", [B, T, 128], BF16, isOutput=False)
    maskA = nc.declare_dram_parameter("maskA", [128, MCOLS], BF16, isOutput=False)
    maskB = nc.declare_dram_parameter("maskB", [128, MCOLS], BF16, isOutput=False)
    flagB = nc.declare_dram_parameter("flagB", [128, 1], mybir.dt.uint8, isOutput=False)
    flagF = nc.declare_dram_parameter("flagF", [128, 1], F32, isOutput=False)
    onesP = nc.declare_dram_parameter("onesP", [128, 1], F32R, isOutput=False)
    flagI = nc.declare_dram_parameter("flagI", [128, 1], F32, isOutput=False)

    y_out = nc.declare_dram_parameter("y_out", [B, C, T], F32, isOutput=True)
    curk = nc.declare_dram_parameter("curk", [B, 2, 128, T], F32R, isOutput=True)
    curv = nc.declare_dram_parameter("curv", [B, 2, T, 128], F32, isOutput=True)

    with tile.TileContext(nc) as tc:
        with (
            tc.tile_pool(name="const", bufs=1) as const,
            tc.tile_pool(name="xwp", bufs=2) as xwp,
            tc.tile_pool(name="qkvp", bufs=2) as qkvp,
            tc.tile_pool(name="attn", bufs=1) as attnp,
            tc.tile_pool(name="ptp", bufs=2) as ptp,
            tc.tile_pool(name="yw", bufs=2) as ywp,
            tc.tile_pool(name="prevp", bufs=1) as prevp,
            tc.tile_pool(name="ytp", bufs=1) as ytp,
            tc.tile_pool(name="ps_a", bufs=3, space="PSUM") as ps_a,
            tc.tile_pool(name="ps_y", bufs=2, space="PSUM") as ps_y,
            tc.tile_pool(name="ps_v", bufs=1, space="PSUM") as ps_v,
            tc.tile_pool(name="ps_d", bufs=2, space="PSUM") as ps_d,
        ):
            # ---- persistent loads ----
            wq_sb = const.tile([128, CK, 6, 128], BF16)
            nc.sync.dma_start(
                wq_sb[:], wqkv.rearrange("(ck p) s d -> p ck s d", p=128)
            )
            wp_sb = const.tile([128, 2, C], BF16)
            nc.sync.dma_start(wp_sb[:], wproj.rearrange("h p o -> p h o"))
            mA_sb = const.tile([128, MCOLS], BF16)
            nc.sync.dma_start(mA_sb[:], maskA[:, :])
            mB_sb = const.tile([128, MCOLS], BF16)
            nc.sync.dma_start(mB_sb[:], maskB[:, :])
            flag_sb = const.tile([128, 1], mybir.dt.uint8)
            nc.sync.dma_start(flag_sb[:], flagB[:, :])
            flagf_sb = const.tile([128, 1], F32)
            nc.sync.dma_start(flagf_sb[:], flagF[:, :])
            flagi_sb = const.tile([128, 1], F32)
            nc.sync.dma_start(flagi_sb[:], flagI[:, :])
            onesM = const.tile([128, 128], BF16)
            nc.vector.memset(onesM[:], 1.0)

            masks = (mA_sb, mB_sb)

            for b in range(B):
                # ---- QKV projection for batch b ----
                qT = qkvp.tile([128, 2, T], F32R, tag="qT")
                kT = qkvp.tile([128, 2, T], F32R, tag="kT")
                vat = [
                    qkvp.tile([128, TK, 128], BF16, tag=f"v{s}", name=f"vat{s}")
                    for s in (0, 1)
                ]
                for tw in range(NW):
                    xw = xwp.tile([128, CK, W], BF16, tag="xw")
                    nc.sync.dma_start(
                        xw[:],
                        xT[b].rearrange("(ck p) t -> p ck t", p=128)[
                            :, :, tw * W : (tw + 1) * W
                        ],
                    )
                    # q/k feature-major: out [f 128, t W]
                    for s in range(4):  # qA qB kA kB
                        acc = ps_a.tile([128, W], F32, tag="mm512")
                        for ck in range(CK):
                            nc.tensor.matmul(
                                acc[:],
                                wq_sb[:, ck, s, :],
                                xw[:, ck, :],
                                start=(ck == 0),
                                stop=(ck == CK - 1),
                            )
                        dst = qT if s < 2 else kT
                        nc.vector.tensor_copy(
                            dst[:, s % 2, tw * W : (tw + 1) * W], acc[:]
                        )
                    # v token-major: out [t 128, (slot, d) 256]
                    for sub in range(W // 128):
                        ckg = tw * (W // 128) + sub
                        accv = ps_v.tile([128, 2, 128], F32, tag="vmm")
                        for ck in range(CK):
                            nc.tensor.matmul(
                                accv[:],
                                xw[:, ck, sub * 128 : (sub + 1) * 128],
                                wq_sb[:, ck, 4:6, :],
                                start=(ck == 0),
                                stop=(ck == CK - 1),
                            )
                        vstage = ywp.tile([128, 2, 128], F32, tag="ystage")
                        for s in (0, 1):
                            nc.scalar.activation(
                                vat[s][:, ckg, :], accv[:, s, :], AF.Copy
                            )
                            nc.vector.tensor_copy(vstage[:, s, :], accv[:, s, :])
                            nc.sync.dma_start(
                                curv[b, s, ckg * 128 : (ckg + 1) * 128, :],
                                vstage[:, s, :],
                            )

                # cur_k out (feature-major; host transposes)
                for s in (0, 1):
                    nc.sync.dma_start(curk[b, s], kT[:, s, :])

                # ---- slot-B attention k/v select (self proj vs prev cache) ----
                pk_sb = prevp.tile([128, T], BF16, tag="pk")
                nc.sync.dma_start(pk_sb[:], prevKT[b])
                # kT_B = kT_B * (1-flag) + prev * flag   (fp32r-safe blend)
                nc.vector.tensor_scalar_mul(kT[:, 1, :], kT[:, 1, :], flagi_sb[:])
                nc.vector.tensor_scalar_mul(pk_sb[:], pk_sb[:], flagf_sb[:])
                nc.vector.tensor_tensor(
                    kT[:, 1, :], kT[:, 1, :], pk_sb[:], mybir.AluOpType.add
                )
                pv_sb = prevp.tile([128, TK, 128], BF16, tag="pv")
                nc.sync.dma_start(
                    pv_sb[:], prevV[b].rearrange("(ck p) d -> p ck d", p=128)
                )
                nc.vector.copy_predicated(
                    vat[1][:], flag_sb[:].to_broadcast((128, TK, 128)), pv_sb[:]
                )

                # ---- attention + projection, per window ----
                yT = ytp.tile([128, 2, T], BF16, tag="yT")
                for wp2 in range(NW // 2):
                  for w in (2 * wp2, 2 * wp2 + 1):
                    yTw = yT[:, :, w * W : (w + 1) * W]
                    for slot in (0, 1):
                        nch = (4 * w + 4) if slot == 0 else TK
                        pT = ptp.tile([128, TK, W], BF16, tag="pT")
                        for j in range(nch):
                            accs = ps_a.tile([128, W], F32, tag="mm512")
                            nc.tensor.matmul(
                                accs[:],
                                kT[:, slot, j * 128 : (j + 1) * 128],
                                qT[:, slot, w * W : (w + 1) * W],
                                start=True,
                                stop=True,
                            )
                            nc.scalar.activation(pT[:, j, :], accs[:], AF.Exp)
                            r = min(max(j - 4 * w, -1), 4)
                            if r >= 0:
                                off = 512 - 128 * r
                                nc.vector.tensor_tensor(
                                    pT[:, j, :],
                                    pT[:, j, :],
                                    masks[slot][:, off : off + W],
                                    mybir.AluOpType.mult,
                                )
                        # PV and denominator accumulated together:
                        # den rows are all equal to the masked-exp column sums
                        accy = ps_y.tile([128, W], F32, tag="pv512")
                        dend = ps_d.tile([128, W], F32, tag="den")
                        for j in range(nch):
                            nc.tensor.matmul(
                                dend[:],
                                onesM[:],
                                pT[:, j, :],
                                start=(j == 0),
                                stop=(j == nch - 1),
                            )
                            nc.tensor.matmul(
                                accy[:],
                                vat[slot][:, j, :],
                                pT[:, j, :],
                                start=(j == 0),
                                stop=(j == nch - 1),
                            )
                        recipb = attnp.tile([128, W], F32, tag="recipb")
                        nc.vector.reciprocal(recipb[:], dend[:])
                        nc.vector.tensor_tensor(
                            yTw[:, slot, :], accy[:], recipb[:], mybir.AluOpType.mult
                        )

                  # output projection for this window pair, feature-major out
                  for oc in range(C // 128):
                        accps = [
                            ps_a.tile([128, W], F32, tag="mm512", name=f"accp{i}")
                            for i in range(2)
                        ]
                        for slot in (0, 1):
                            for i in range(2):
                                w = wp2 * 2 + i
                                nc.tensor.matmul(
                                    accps[i][:],
                                    wp_sb[:, slot, oc * 128 : (oc + 1) * 128],
                                    yT[:, slot, w * W : (w + 1) * W],
                                    start=(slot == 0),
                                    stop=(slot == 1),
                                )
                        for i in range(2):
                            w = wp2 * 2 + i
                            ystage = ywp.tile([128, W], F32, tag="ystage")
                            nc.vector.tensor_copy(ystage[:], accps[i][:])
                            nc.sync.dma_start(
                                y_out[b, oc * 128 : (oc + 1) * 128,
                                      w * W : (w + 1) * W],
                                ystage[:],
                            )
    nc.compile()
    return nc


_CACHE = {}


def _get_nc():
    if "nc" not in _CACHE:
        nc = bacc.Bacc(None, target_bir_lowering=False, debug=False)
        _CACHE["nc"] = _emit(nc)
    return _CACHE["nc"]


def _host_inputs(x, w_attn, w_proj, prevs):
    """Build the 8 per-core input maps."""
    xTh = np.ascontiguousarray(x.transpose(0, 2, 1)).astype(ml_dtypes.bfloat16)
    scale = np.float32(1.0 / np.sqrt(D))

    kk = np.arange(128)[:, None]
    cc = np.arange(MCOLS)[None, :]
    stair = (cc >= kk + 512).astype(ml_dtypes.bfloat16)
    ones_m = np.ones((128, MCOLS), dtype=ml_dtypes.bfloat16)
    zkt = np.zeros((B, 128, T), dtype=ml_dtypes.bfloat16)
    zv = np.zeros((B, T, 128), dtype=ml_dtypes.bfloat16)

    in_maps = []
    for c in range(N_CORES):
        hA, hB = _core_heads(c)
        cols = []
        for h in (hA, hB):
            cols.append(w_attn[h * D : (h + 1) * D, :].T * scale)  # q (scaled)
        for h in (hA, hB):
            cols.append(w_attn[C + h * D : C + (h + 1) * D, :].T)  # k
        for h in (hA, hB):
            cols.append(w_attn[2 * C + h * D : 2 * C + (h + 1) * D, :].T)  # v
        # order: qA qB kA kB vA vB -> [C, 6, 128]
        wq = np.stack(cols, axis=1).astype(ml_dtypes.bfloat16)
        wp = np.stack(
            [w_proj[:, hA * D : (hA + 1) * D].T, w_proj[:, hB * D : (hB + 1) * D].T],
            axis=0,
        ).astype(ml_dtypes.bfloat16)  # [2, 128, C]

        cross = c >= 4
        if cross:
            i = hB - SA
            pk, pv = prevs[i]
            pkt = np.ascontiguousarray(pk[:, hB].transpose(0, 2, 1)).astype(
                ml_dtypes.bfloat16
            )  # [B,128,T]
            pvv = np.ascontiguousarray(pv[:, hB]).astype(ml_dtypes.bfloat16)
        else:
            pkt, pvv = zkt, zv

        in_maps.append(
            {
                "xT": xTh,
                "wqkv": np.ascontiguousarray(wq),
                "wproj": np.ascontiguousarray(wp),
                "prevKT": pkt,
                "prevV": pvv,
                "maskA": stair,
                "maskB": ones_m if cross else stair,
                "flagB": np.full((128, 1), 1 if cross else 0, dtype=np.uint8),
                "flagF": np.full((128, 1), 1.0 if cross else 0.0, dtype=np.float32),
                "onesP": np.ones((128, 1), dtype=np.float32),
                "flagI": np.full((128, 1), 0.0 if cross else 1.0, dtype=np.float32),
            }
        )
    return in_maps


def kernel(x, w_attn, w_proj,
           prev_k0, prev_v0, prev_k1, prev_v1,
           prev_k2, prev_v2, prev_k3, prev_v3,
           _trace=False):
    x = np.asarray(x, dtype=np.float32)
    w_attn = np.asarray(w_attn, dtype=np.float32)
    w_proj = np.asarray(w_proj, dtype=np.float32)
    prevs = [
        (np.asarray(prev_k0), np.asarray(prev_v0)),
        (np.asarray(prev_k1), np.asarray(prev_v1)),
        (np.asarray(prev_k2), np.asarray(prev_v2)),
        (np.asarray(prev_k3), np.asarray(prev_v3)),
    ]
    nc = _get_nc()
    in_maps = _host_inputs(x, w_attn, w_proj, prevs)
    res = run_bass_kernel_spmd(
        nc, in_maps, core_ids=list(range(N_CORES)), trace=_trace
    )
    kernel.last_exec_time_ns = res.exec_time_ns

    y = np.zeros((B, C, T), dtype=np.float64)
    cur_k = np.zeros((B, H, T, D), dtype=np.float32)
    cur_v = np.zeros((B, H, T, D), dtype=np.float32)
    for c in range(N_CORES):
        out = res.results[c]
        y += out["y_out"].astype(np.float64)
        hA, hB = _core_heads(c)
        for s, h in ((0, hA), (1, hB)):
            cur_k[:, h] = out["curk"][:, s].transpose(0, 2, 1)
            cur_v[:, h] = out["curv"][:, s]
    return np.ascontiguousarray(y.transpose(0, 2, 1)).astype(np.float32), cur_k, cur_v


kernel.last_exec_time_ns = None


# revision 23
# speedup vs baseline: 1.6099x; 1.0220x over previous
"""TRN2 Bass kernel for nn_Attention_79628693668242 (sparse attention).

Head-parallel (tensor parallel) across 8 NeuronCores, 2 heads per core:
  cores 0-3: heads (2c, 2c+1)            -- both causal self-attention
  cores 4-7: heads (c+4, c+8)            -- one self head + one cross head

Uniform SPMD program; per-core behavior differs only through data:
  - mask tensors (causal staircase vs all-ones)
  - a flag that predicate-selects slot-B attention k/v between the current
    projection (self) and the prev-layer cache (cross)

All matmul operands are kept in layouts that need no transposes:
  - x is host-transposed to xT [B, C, T] (fp32r)
  - q,k projected feature-major [d, t]; v projected token-major [t, d]
  - PV computes yT = v.T @ pT feature-major, which directly feeds the
    output projection as lhsT; proj emits token-major y tiles.
Softmax skips the max-subtraction (scores are bounded ~|6| here, exp is
safe in fp32) which makes the whole softmax free-dim-local except for the
denominator: S = sum_chunks exp-chunks (DVE), column sums via a ones
matmul, reciprocal, gpsimd partition-broadcast, folded into the PV-psum
eviction multiply.
"""

import numpy as np
import ml_dtypes

import concourse.bass as bass
import concourse.tile as tile
from concourse import bacc, mybir
from concourse.bass_utils import run_bass_kernel_spmd

F32 = mybir.dt.float32
F32R = mybir.dt.float32r
BF16 = mybir.dt.bfloat16
AF = mybir.ActivationFunctionType

B, T, C, D = 4, 2048, 2048, 128
H, SA = 16, 12
CK = C // 128          # 16 contraction chunks
TK = T // 128          # 16 token chunks per batch
W = 512                # q / t window
NW = T // W            # 4 windows per batch
MCOLS = 1152           # mask columns: slice offsets 0..640

N_CORES = 8


def _core_heads(c):
    if c < 4:
        return 2 * c, 2 * c + 1
    return c + 4, c + 8


def _emit(nc):
    xT = nc.declare_dram_parameter("xT", [B, C, T], BF16, isOutput=False)
    wqkv = nc.declare_dram_parameter("wqkv", [C, 6, 128], BF16, isOutput=False)
    wproj = nc.declare_dram_parameter("wproj", [2, 128, C], BF16, isOutput=False)
    prevKT = nc.declare_dram_parameter("prevKT", [B, 128, T], BF16, isOutput=False)
    prevV = nc.declare_dram_parameter("prevV", [B, T, 128], BF16, isOutput=False)
    maskA = nc.declare_dram_parameter("maskA", [128, MCOLS], BF16, isOutput=False)
    maskB = nc.declare_dram_parameter("maskB", [128, MCOLS], BF16, isOutput=False)
    flagB = nc.declare_dram_parameter("flagB", [128, 1], mybir.dt.uint8, isOutput=False)
    flagF = nc.declare_dram_parameter("flagF", [128, 1], F32, isOutput=False)
    onesP = nc.declare_dram_parameter("onesP", [128, 1], F32R, isOutput=False)
    flagI = nc.declare_dram_parameter("flagI", [128, 1], F32, isOutput=False)

    y_out = nc.declare_dram_parameter("y_out", [B, C, T], F32, isOutput=True)
    curk = nc.declare_dram_parameter("curk", [B, 2, 128, T], F32R, isOutput=True)
    curv = nc.declare_dram_parameter("curv", [B, 2, T, 128], F32, isOutput=True)

    with tile.TileContext(nc) as tc:
        with (
            tc.tile_pool(name="const", bufs=1) as const,
            tc.tile_pool(name="xwp", bufs=2) as xwp,
            tc.tile_pool(name="qkvp", bufs=2) as qkvp,
            tc.tile_pool(name="attn", bufs=1) as attnp,
            tc.tile_pool(name="ptp", bufs=2) as ptp,
            tc.tile_pool(name="yw", bufs=2) as ywp,
            tc.tile_pool(name="prevp", bufs=1) as prevp,
            tc.tile_pool(name="ytp", bufs=1) as ytp,
            tc.tile_pool(name="ps_a", bufs=3, space="PSUM") as ps_a,
            tc.tile_pool(name="ps_y", bufs=2, space="PSUM") as ps_y,
            tc.tile_pool(name="ps_v", bufs=1, space="PSUM") as ps_v,
            tc.tile_pool(name="ps_d", bufs=2, space="PSUM") as ps_d,
        ):
            # ---- persistent loads ----
            wq_sb = const.tile([128, CK, 6, 128], BF16)
            nc.sync.dma_start(
                wq_sb[:], wqkv.rearrange("(ck p) s d -> p ck s d", p=128)
            )
            wp_sb = const.tile([128, 2, C], BF16)
            nc.sync.dma_start(wp_sb[:], wproj.rearrange("h p o -> p h o"))
            mA_sb = const.tile([128, MCOLS], BF16)
            nc.sync.dma_start(mA_sb[:], maskA[:, :])
            mB_sb = const.tile([128, MCOLS], BF16)
            nc.sync.dma_start(mB_sb[:], maskB[:, :])
            flag_sb = const.tile([128, 1], mybir.dt.uint8)
            nc.sync.dma_start(flag_sb[:], flagB[:, :])
            flagf_sb = const.tile([128, 1], F32)
            nc.sync.dma_start(flagf_sb[:], flagF[:, :])
            flagi_sb = const.tile([128, 1], F32)
            nc.sync.dma_start(flagi_sb[:], flagI[:, :])
            onesM = const.tile([128, 128], BF16)
            nc.vector.memset(onesM[:], 1.0)

            masks = (mA_sb, mB_sb)

            for b in range(B):
                # ---- QKV projection for batch b ----
                qT = qkvp.tile([128, 2, T], F32R, tag="qT")
                kT = qkvp.tile([128, 2, T], F32R, tag="kT")
                vat = [
                    qkvp.tile([128, TK, 128], BF16, tag=f"v{s}", name=f"vat{s}")
                    for s in (0, 1)
                ]
                for tw in range(NW):
                    xw = xwp.tile([128, CK, W], BF16, tag="xw")
                    xsrc = xT[b].rearrange("(ck p) t -> p ck t", p=128)[
                        :, :, tw * W : (tw + 1) * W
                    ]
                    nc.sync.dma_start(xw[:, : CK // 2, :], xsrc[:, : CK // 2, :])
                    nc.sync.dma_start(xw[:, CK // 2 :, :], xsrc[:, CK // 2 :, :])
                    # q/k feature-major: out [f 128, t W]
                    for s in range(4):  # qA qB kA kB
                        acc = ps_a.tile([128, W], F32, tag="mm512")
                        for ck in range(CK):
                            nc.tensor.matmul(
                                acc[:],
                                wq_sb[:, ck, s, :],
                                xw[:, ck, :],
                                start=(ck == 0),
                                stop=(ck == CK - 1),
                            )
                        dst = qT if s < 2 else kT
                        nc.vector.tensor_copy(
                            dst[:, s % 2, tw * W : (tw + 1) * W], acc[:]
                        )
                    # v token-major: out [t 128, (slot, d) 256]
                    for sub in range(W // 128):
                        ckg = tw * (W // 128) + sub
                        accv = ps_v.tile([128, 2, 128], F32, tag="vmm")
                        for ck in range(CK):
                            nc.tensor.matmul(
                                accv[:],
                                xw[:, ck, sub * 128 : (sub + 1) * 128],
                                wq_sb[:, ck, 4:6, :],
                                start=(ck == 0),
                                stop=(ck == CK - 1),
                            )
                        vstage = ywp.tile([128, 2, 128], F32, tag="ystage")
                        for s in (0, 1):
                            nc.scalar.activation(
                                vat[s][:, ckg, :], accv[:, s, :], AF.Copy
                            )
                            nc.vector.tensor_copy(vstage[:, s, :], accv[:, s, :])
                            nc.sync.dma_start(
                                curv[b, s, ckg * 128 : (ckg + 1) * 128, :],
                                vstage[:, s, :],
                            )

                # cur_k out (feature-major; host transposes)
                for s in (0, 1):
                    nc.sync.dma_start(curk[b, s], kT[:, s, :])

                # ---- slot-B attention k/v select (self proj vs prev cache) ----
                pk_sb = prevp.tile([128, T], BF16, tag="pk")
                nc.sync.dma_start(pk_sb[:], prevKT[b])
                # kT_B = kT_B * (1-flag) + prev * flag   (fp32r-safe blend)
                nc.vector.tensor_scalar_mul(kT[:, 1, :], kT[:, 1, :], flagi_sb[:])
                nc.vector.tensor_scalar_mul(pk_sb[:], pk_sb[:], flagf_sb[:])
                nc.vector.tensor_tensor(
                    kT[:, 1, :], kT[:, 1, :], pk_sb[:], mybir.AluOpType.add
                )
                pv_sb = prevp.tile([128, TK, 128], BF16, tag="pv")
                nc.sync.dma_start(
                    pv_sb[:], prevV[b].rearrange("(ck p) d -> p ck d", p=128)
                )
                nc.vector.copy_predicated(
                    vat[1][:], flag_sb[:].to_broadcast((128, TK, 128)), pv_sb[:]
                )

                # ---- attention + projection, per window ----
                yT = ytp.tile([128, 2, T], BF16, tag="yT")
                for wp2 in range(NW // 2):
                  for w in (2 * wp2, 2 * wp2 + 1):
                    yTw = yT[:, :, w * W : (w + 1) * W]
                    for slot in (0, 1):
                        nch = (4 * w + 4) if slot == 0 else TK
                        pT = ptp.tile([128, TK, W], BF16, tag="pT")
                        for j in range(nch):
                            accs = ps_a.tile([128, W], F32, tag="mm512")
                            nc.tensor.matmul(
                                accs[:],
                                kT[:, slot, j * 128 : (j + 1) * 128],
                                qT[:, slot, w * W : (w + 1) * W],
                                start=True,
                                stop=True,
                            )
                            nc.scalar.activation(pT[:, j, :], accs[:], AF.Exp)
                            r = min(max(j - 4 * w, -1), 4)
                            if r >= 0:
                                off = 512 - 128 * r
                                nc.vector.tensor_tensor(
                                    pT[:, j, :],
                                    pT[:, j, :],
                                    masks[slot][:, off : off + W],
                                    mybir.AluOpType.mult,
                                )
                        # PV and denominator accumulated together:
                        # den rows are all equal to the masked-exp column sums
                        accy = ps_y.tile([128, W], F32, tag="pv512")
                        dend = ps_d.tile([128, W], F32, tag="den")
                        for j in range(nch):
                            nc.tensor.matmul(
                                dend[:],
                                onesM[:],
                                pT[:, j, :],
                                start=(j == 0),
                                stop=(j == nch - 1),
                            )
                            nc.tensor.matmul(
                                accy[:],
                                vat[slot][:, j, :],
                                pT[:, j, :],
                                start=(j == 0),
                                stop=(j == nch - 1),
                            )
                        recipb = attnp.tile([128, W], F32, tag="recipb")
                        nc.vector.reciprocal(recipb[:], dend[:])
                        nc.vector.tensor_tensor(
                            yTw[:, slot, :], accy[:], recipb[:], mybir.AluOpType.mult
                        )

                  # output projection for this window pair, feature-major out
                  for oc in range(C // 128):
                        accps = [
                            ps_a.tile([128, W], F32, tag="mm512", name=f"accp{i}")
                            for i in range(2)
                        ]
                        for slot in (0, 1):
                            for i in range(2):
                                w = wp2 * 2 + i
                                nc.tensor.matmul(
                                    accps[i][:],
                                    wp_sb[:, slot, oc * 128 : (oc + 1) * 128],
                                    yT[:, slot, w * W : (w + 1) * W],
                                    start=(slot == 0),
                                    stop=(slot == 1),
                                )
                        for i in range(2):
                            w = wp2 * 2 + i
                            ystage = ywp.tile([128, W], F32, tag="ystage")
                            nc.vector.tensor_copy(ystage[:], accps[i][:])
                            nc.sync.dma_start(
                                y_out[b, oc * 128 : (oc + 1) * 128,
                                      w * W : (w + 1) * W],
                                ystage[:],
                            )
    nc.compile()
    return nc


_CACHE = {}


def _get_nc():
    if "nc" not in _CACHE:
        nc = bacc.Bacc(None, target_bir_lowering=False, debug=False)
        _CACHE["nc"] = _emit(nc)
    return _CACHE["nc"]


def _host_inputs(x, w_attn, w_proj, prevs):
    """Build the 8 per-core input maps."""
    xTh = np.ascontiguousarray(x.transpose(0, 2, 1)).astype(ml_dtypes.bfloat16)
    scale = np.float32(1.0 / np.sqrt(D))

    kk = np.arange(128)[:, None]
    cc = np.arange(MCOLS)[None, :]
    stair = (cc >= kk + 512).astype(ml_dtypes.bfloat16)
    ones_m = np.ones((128, MCOLS), dtype=ml_dtypes.bfloat16)
    zkt = np.zeros((B, 128, T), dtype=ml_dtypes.bfloat16)
    zv = np.zeros((B, T, 128), dtype=ml_dtypes.bfloat16)

    in_maps = []
    for c in range(N_CORES):
        hA, hB = _core_heads(c)
        cols = []
        for h in (hA, hB):
            cols.append(w_attn[h * D : (h + 1) * D, :].T * scale)  # q (scaled)
        for h in (hA, hB):
            cols.append(w_attn[C + h * D : C + (h + 1) * D, :].T)  # k
        for h in (hA, hB):
            cols.append(w_attn[2 * C + h * D : 2 * C + (h + 1) * D, :].T)  # v
        # order: qA qB kA kB vA vB -> [C, 6, 128]
        wq = np.stack(cols, axis=1).astype(ml_dtypes.bfloat16)
        wp = np.stack(
            [w_proj[:, hA * D : (hA + 1) * D].T, w_proj[:, hB * D : (hB + 1) * D].T],
            axis=0,
        ).astype(ml_dtypes.bfloat16)  # [2, 128, C]

        cross = c >= 4
        if cross:
            i = hB - SA
            pk, pv = prevs[i]
            pkt = np.ascontiguousarray(pk[:, hB].transpose(0, 2, 1)).astype(
                ml_dtypes.bfloat16
            )  # [B,128,T]
            pvv = np.ascontiguousarray(pv[:, hB]).astype(ml_dtypes.bfloat16)
        else:
            pkt, pvv = zkt, zv

        in_maps.append(
            {
                "xT": xTh,
                "wqkv": np.ascontiguousarray(wq),
                "wproj": np.ascontiguousarray(wp),
                "prevKT": pkt,
                "prevV": pvv,
                "maskA": stair,
                "maskB": ones_m if cross else stair,
                "flagB": np.full((128, 1), 1 if cross else 0, dtype=np.uint8),
                "flagF": np.full((128, 1), 1.0 if cross else 0.0, dtype=np.float32),
                "onesP": np.ones((128, 1), dtype=np.float32),
                "flagI": np.full((128, 1), 0.0 if cross else 1.0, dtype=np.float32),
            }
        )
    return in_maps


def kernel(x, w_attn, w_proj,
           prev_k0, prev_v0, prev_k1, prev_v1,
           prev_k2, prev_v2, prev_k3, prev_v3,
           _trace=False):
    x = np.asarray(x, dtype=np.float32)
    w_attn = np.asarray(w_attn, dtype=np.float32)
    w_proj = np.asarray(w_proj, dtype=np.float32)
    prevs = [
        (np.asarray(prev_k0), np.asarray(prev_v0)),
        (np.asarray(prev_k1), np.asarray(prev_v1)),
        (np.asarray(prev_k2), np.asarray(prev_v2)),
        (np.asarray(prev_k3), np.asarray(prev_v3)),
    ]
    nc = _get_nc()
    in_maps = _host_inputs(x, w_attn, w_proj, prevs)
    res = run_bass_kernel_spmd(
        nc, in_maps, core_ids=list(range(N_CORES)), trace=_trace
    )
    kernel.last_exec_time_ns = res.exec_time_ns

    y = np.zeros((B, C, T), dtype=np.float64)
    cur_k = np.zeros((B, H, T, D), dtype=np.float32)
    cur_v = np.zeros((B, H, T, D), dtype=np.float32)
    for c in range(N_CORES):
        out = res.results[c]
        y += out["y_out"].astype(np.float64)
        hA, hB = _core_heads(c)
        for s, h in ((0, hA), (1, hB)):
            cur_k[:, h] = out["curk"][:, s].transpose(0, 2, 1)
            cur_v[:, h] = out["curv"][:, s]
    return np.ascontiguousarray(y.transpose(0, 2, 1)).astype(np.float32), cur_k, cur_v


kernel.last_exec_time_ns = None
